# revision 1
# baseline (speedup 1.0000x reference)
"""Trainium2 Bass kernel for AttentionM (dense transformer block).

Computes, for x [4, 2048, 1024] and q/k/v CSS-gated projections:
    q = (x@Wq+bq)*sigmoid(x@Wqc+bqc)   -> [B, Sp, 16 heads, 16]
    k = likewise                        -> [B, Sp, 16, 16]
    v = likewise (64-wide heads)        -> [B, Sp, 16, 64]
    ctx = softmax(q k^T / 8) v          -> [B, S, 1024]
with Sp = S+16 zero-padded rows (pad tokens participate via bias-only css).

Sharding over 8 NeuronCores: 4-way data parallel over batch x 2-way tensor
parallel over heads (8 heads per core). Each core gets x[b] (padded) and its
head-slice of the weights, computes ctx[b, :, hg*512:(hg+1)*512].

Per-core dataflow (all matmuls in float32r, 1 cycle/row at N>=256):
  1. x streams in four 512-column blocks; each block is PE-transposed to a
     column-range xT part, immediately feeding that block's kT/qT projection
     chunks (feature-major [128 = 8h x 16, seq], sigmoid gate fused via one
     ACT op + one DVE scalar_tensor_tensor) while later blocks are still in
     flight from HBM. v tiles (token-major [tok, 8, 64+1], bias added via a
     K=1 ones-column matmul) lag one block behind so the PE never waits.
  2. The 16 identical zero-pad k rows collapse into one rank-1 update:
     block-diag pad-k [128, 8] batches all heads' pad scores into two
     [8, 1024] matmuls and exps (ln 16 folded into the ACT bias), restaged to
     partition 0. v'_pad = bv*sigmoid(bvc) directly (pad x rows are zero).
  3. Attention per (qr in 2, h in 8): scoresT [k_tile, q] = kh^T qh on PE
     (kh/qh restaged to partition 0 by SBUF->SBUF DMA -- matmul operands must
     share a 32-aligned base partition); exp on ACT straight out of PSUM
     (scale=1/8, no max subtraction -- scores are provably in [-3, 3]);
     ctxT [65, q] accumulated over k tiles in PSUM, with row 64 = exp @ ones
     as the softmax denominator. The PSUM group is opened by the dep-free pad
     rank-1 update and the ctx matmuls run three k-tiles behind the scores, so
     the ACT exp stream (the bottleneck: 1038ns per [128,1024] exp) runs
     back-to-back.
  4. Epilogue per head (deferred into the next head's k-loop): PE-transpose
     ctxT to token-major, DVE reciprocal of the denominator column, multiply,
     and merged output DMAs (heads 0..6 flush during the last head's loop).
"""

import sys

if "/opt/trn_rl_repo" not in sys.path:
    sys.path.insert(0, "/opt/trn_rl_repo")

import numpy as np

import concourse.bacc as bacc
import concourse.mybir as mybir
import concourse.tile as tile
from concourse.bass_utils import run_bass_kernel_spmd
from concourse.masks import make_identity

F32 = mybir.dt.float32
F32R = mybir.dt.float32r
AF = mybir.ActivationFunctionType
ALU = mybir.AluOpType

B = 4
S = 2048          # real sequence
PAD = 16
SP = S + PAD      # padded sequence (k extent)
D = 1024
DC = D // 128     # 8 contraction chunks
HL = 8            # heads per core
QL = 16           # q/k head dim
VL = 64           # v head dim
NKT = SP // 128 + 1          # 17 k tiles (16 full + 16-row tail)
QR = 1024                    # q range per psum accumulator
SCALE = 1.0 / 8.0            # 1/sqrt(64)


def _build(repeat=1):
    nc = bacc.Bacc("TRN2", target_bir_lowering=False, debug=False, num_devices=8)

    x_d = nc.dram_tensor("x", [SP, D], F32R, kind="ExternalInput").ap()
    idr_d = nc.dram_tensor("idr", [128, 128], F32R, kind="ExternalInput").ap()
    bdiag_d = nc.dram_tensor("bdiag", [128, 8], F32R, kind="ExternalInput").ap()
    wq_d = nc.dram_tensor("wq", [D, 128], F32R, kind="ExternalInput").ap()
    wqc_d = nc.dram_tensor("wqc", [D, 128], F32R, kind="ExternalInput").ap()
    wk_d = nc.dram_tensor("wk", [D, 128], F32R, kind="ExternalInput").ap()
    wkc_d = nc.dram_tensor("wkc", [D, 128], F32R, kind="ExternalInput").ap()
    wv_d = nc.dram_tensor("wv", [D, 512], F32R, kind="ExternalInput").ap()
    wvc_d = nc.dram_tensor("wvc", [D, 512], F32R, kind="ExternalInput").ap()
    bq_d = nc.dram_tensor("bq", [128], F32, kind="ExternalInput").ap()
    bqc_d = nc.dram_tensor("bqc", [128], F32, kind="ExternalInput").ap()
    bk_d = nc.dram_tensor("bk", [128], F32, kind="ExternalInput").ap()
    bkc_d = nc.dram_tensor("bkc", [128], F32, kind="ExternalInput").ap()
    bv_d = nc.dram_tensor("bv", [512], F32R, kind="ExternalInput").ap()
    bvc_d = nc.dram_tensor("bvc", [512], F32R, kind="ExternalInput").ap()
    y_d = nc.dram_tensor("y", [S, 512], F32, kind="ExternalOutput").ap()

    with tile.TileContext(nc) as tc:
        for _ in range(repeat):
            _emit(nc, tc, x_d, idr_d, bdiag_d, wq_d, wqc_d, wk_d, wkc_d, wv_d,
                  wvc_d, bq_d, bqc_d, bk_d, bkc_d, bv_d, bvc_d, y_d)
    nc.compile()
    return nc


def _emit(nc, tc, x_d, idr_d, bdiag_d, wq_d, wqc_d, wk_d, wkc_d, wv_d,
          wvc_d, bq_d, bqc_d, bk_d, bkc_d, bv_d, bvc_d, y_d):
    # ---------------- long-lived pools ----------------
    const = tc.alloc_tile_pool(name="const", bufs=1)
    proj = tc.alloc_tile_pool(name="proj", bufs=1)
    padp = tc.alloc_tile_pool(name="padp", bufs=1)
    pp_mm = tc.alloc_tile_pool(name="pp_mm", bufs=2, space="PSUM")
    pp_tp = tc.alloc_tile_pool(name="pp_tp", bufs=2, space="PSUM")
    pp_acc = tc.alloc_tile_pool(name="pp_acc", bufs=1, space="PSUM")

    ph13 = tc.alloc_tile_pool(name="ph13", bufs=1)
    # xT split by column range so consumers start before all of x is transposed
    xT_parts = [ph13.tile([128, DC, 528 if i == 3 else 512], F32R, name=f"xT{i}")
                for i in range(4)]

    def xTs(d, c0, csz):
        part = min(c0 // 512, 3)
        lo = c0 - part * 512
        assert lo + csz <= (528 if part == 3 else 512)
        return xT_parts[part][:, d, lo:lo + csz]
    wpool = tc.alloc_tile_pool(name="wpool", bufs=1)
    ph1 = tc.alloc_tile_pool(name="ph1", bufs=4)

    # f32r identity first: the x transposes block on it
    idr = const.tile([128, 128], F32R, name="idr")
    nc.sync.dma_start(out=idr, in_=idr_d)
    ident = const.tile([128, 128], F32)
    make_identity(nc, ident)

    # ---- phases 1+2 interleaved: x-block transposes feed q/k chunks ----
    # Block b = x tiles 4b..4b+3 (block 3 also takes the 16-row pad tile).
    # After block b's transposes, the kT and qT projection chunks for column
    # range [512b, 512(b+1)) are emitted, so the PE works on projections while
    # the next x block is still streaming from HBM.
    def ppart(bias_d, dtype=F32):
        t = const.tile([128, 1], dtype, name=f"b_{bias_d.name}")
        nc.sync.dma_start(out=t, in_=bias_d.unsqueeze(-1))
        return t

    wq = wpool.tile([128, DC, 128], F32R, name="wq")
    wqc = wpool.tile([128, DC, 128], F32R, name="wqc")
    wk = wpool.tile([128, DC, 128], F32R, name="wk")
    wkc = wpool.tile([128, DC, 128], F32R, name="wkc")
    wv = wpool.tile([128, DC, 512], F32R, name="wv")
    wvc = wpool.tile([128, DC, 512], F32R, name="wvc")

    qT = proj.tile([128, S], F32R, name="qT")       # [8h*16, q]
    kT = proj.tile([128, SP], F32R, name="kT")      # [8h*16, k]
    vt = proj.tile([128, NKT, HL, VL + 1], F32R, name="vt")  # token-major v + ones

    ones_col = const.tile([1, 128], F32R, name="ones_col")

    # pre-attention phases rotate a third psum slot through the (idle)
    # attention accumulator pool for deeper pipelining
    _rr = [0]

    def mm_tile():
        _rr[0] += 1
        if _rr[0] % 3 == 0:
            return pp_acc.tile([128, QR], F32, name="acc")
        return pp_mm.tile([128, 1024], F32, name="mm")

    sig2 = tc.alloc_tile_pool(name="sig2", bufs=2)
    sig3 = tc.alloc_tile_pool(name="sig3", bufs=2)
    emit_v_tile_fn = [None]

    def emit_x_tile(t):
        tsz = min(128, SP - t * 128)
        xt = ph1.tile([128, D], F32R, name="xload")
        nc.sync.dma_start(out=xt[0:tsz, :], in_=x_d[t * 128: t * 128 + tsz, :])
        return xt, tsz

    def emit_tposes(t, xt, tsz):
        for half in range(2):
            tp = pp_tp.tile([128, 512], F32R, name="tp")
            for jj in range(4):
                d = half * 4 + jj
                nc.tensor.transpose(
                    out=tp[:, jj * 128: jj * 128 + tsz],
                    in_=xt[0:tsz, d * 128:(d + 1) * 128],
                    identity=idr[0:tsz, 0:tsz],
                )
            part = min(t // 4, 3)
            lo = t * 128 - part * 512
            nc.vector.tensor_copy(
                out=xT_parts[part][:, half * 4:(half + 1) * 4, lo:lo + tsz],
                in_=tp.rearrange("p (b c) -> p b c", b=4)[:, :, 0:tsz],
            )

    def emit_v_tile(t):
        """v'[t] = [(lin+bv) * sigmoid(linc+bvc) | 1], token-major."""
        tsz = min(128, SP - t * 128)
        tc0 = t * 128
        ps = mm_tile()
        for d in range(DC):
            nc.tensor.matmul(ps[0:tsz, 0:512], xTs(d, tc0, tsz), wv[:, d, :],
                             start=(d == 0), stop=False)
        nc.tensor.matmul(ps[0:tsz, 0:512], ones_col[:, 0:tsz], bv_row,
                         start=False, stop=True)
        for d in range(DC):
            nc.tensor.matmul(ps[0:tsz, 512:1024], xTs(d, tc0, tsz),
                             wvc[:, d, :], start=(d == 0), stop=False)
        nc.tensor.matmul(ps[0:tsz, 512:1024], ones_col[:, 0:tsz], bvc_row,
                         start=False, stop=True)
        sg = sig3.tile([128, 512], F32, name="sigv")
        nc.scalar.activation(out=sg[0:tsz, :], in_=ps[0:tsz, 512:1024],
                             func=AF.Sigmoid)
        nc.vector.tensor_tensor(
            out=vt[0:tsz, t, :, 0:VL],
            in0=ps[0:tsz, 0:512].rearrange("p (h v) -> p h v", h=HL),
            in1=sg[0:tsz, :].rearrange("p (h v) -> p h v", h=HL),
            op=ALU.mult)
        nc.scalar.activation(out=vt[:, t, :, VL:VL + 1],
                             in_=idr[:, 0:HL].unsqueeze(-1),
                             func=AF.Copy, scale=0.0, bias=1.0)
    emit_v_tile_fn[0] = emit_v_tile

    def emit_qk_chunk(wl, wcl, bl, bcl, dest, c0, csz):
        ps = mm_tile()
        for d in range(DC):
            nc.tensor.matmul(ps[:, 0:csz], wl[:, d, :], xTs(d, c0, csz),
                             start=(d == 0), stop=(d == DC - 1))
        for d in range(DC):
            nc.tensor.matmul(ps[:, 512:512 + csz], wcl[:, d, :],
                             xTs(d, c0, csz),
                             start=(d == 0), stop=(d == DC - 1))
        sig = sig2.tile([128, 512], F32, name="sig")
        nc.scalar.activation(out=sig[:, 0:csz], in_=ps[:, 512:512 + csz],
                             func=AF.Sigmoid, bias=bcl)
        nc.vector.scalar_tensor_tensor(
            out=dest[:, c0:c0 + csz], in0=ps[:, 0:csz], scalar=bl,
            in1=sig[:, 0:csz], op0=ALU.add, op1=ALU.mult)

    bias_sbs = None
    emitted_v = 0
    for blk in range(4):
        ts_in_blk = range(12, NKT) if blk == 3 else range(blk * 4, blk * 4 + 4)
        for t in ts_in_blk:
            xt, tsz = emit_x_tile(t)
            if bias_sbs is None:
                bq_sb = ppart(bq_d)
                bqc_sb = ppart(bqc_d)
                bk_sb = ppart(bk_d)
                bkc_sb = ppart(bkc_d)
                bv_row = const.tile([1, 512], F32R, name="bv_row")
                nc.sync.dma_start(out=bv_row, in_=bv_d.unsqueeze(0))
                bvc_row = const.tile([1, 512], F32R, name="bvc_row")
                nc.sync.dma_start(out=bvc_row, in_=bvc_d.unsqueeze(0))
                nc.scalar.activation(out=ones_col, in_=idr[0:1, :], func=AF.Copy,
                                     scale=0.0, bias=1.0)
                bias_sbs = True
            emit_tposes(t, xt, tsz)
        if blk == 0:
            # qk weights after the first x block: needed from ~10us on, and
            # they must not delay the x stream that gates the transposes
            for w_sb, w_dd in ((wk, wk_d), (wkc, wkc_d), (wq, wq_d), (wqc, wqc_d)):
                nc.sync.dma_start(out=w_sb,
                                  in_=w_dd.rearrange("(a p) c -> p a c", p=128))
        if blk == 1:
            # v weights after the second x block; v tiles lag one block so
            # these arrive before the first v matmuls need them
            wrv = wv_d.rearrange("(a p) c -> p a c", p=128)
            wrvc = wvc_d.rearrange("(a p) c -> p a c", p=128)
            for d in range(DC):
                nc.sync.dma_start(out=wv[:, d, :], in_=wrv[:, d, :])
                nc.sync.dma_start(out=wvc[:, d, :], in_=wrvc[:, d, :])
        emit_qk_chunk(wk, wkc, bk_sb, bkc_sb, kT, blk * 512, 512)
        emit_qk_chunk(wq, wqc, bq_sb, bqc_sb, qT, blk * 512, 512)
        if blk >= 1:
            while emitted_v < blk * 4:
                emit_v_tile_fn[0](emitted_v)
                emitted_v += 1
    emit_qk_chunk(wk, wkc, bk_sb, bkc_sb, kT, S, PAD)
    # The 16 identical zero-pad k rows collapse into one rank-1 update:
    # acc += exp(s_pad/8 + ln 16) * v'_pad. Build block-diag pad-k [128, 8]
    # (head h's pad-k vector at rows 16h..16h+16), batch all heads' pad
    # scores into two [8, 1024] matmuls + one exp each, then restage to
    # partition 0 for the K=1 ctx update.
    padk = padp.tile([128, HL], F32R, name="padk")
    bdiag = padp.tile([128, HL], F32R, name="bdiag")
    nc.sync.dma_start(out=bdiag, in_=bdiag_d)
    nc.vector.tensor_scalar(out=padk, in0=bdiag,
                            scalar1=kT[:, S:S + 1].bitcast(F32),
                            scalar2=None, op0=ALU.mult)
    e_pad = padp.tile([HL, 2, QR], F32R, name="e_pad")
    LN16 = float(np.log(16.0))
    ln16_sb = padp.tile([128, 1], F32, name="ln16")
    nc.scalar.activation(out=ln16_sb, in_=idr[:, 0:1], func=AF.Copy,
                         scale=0.0, bias=LN16)
    for r in range(2):
        pps = pp_mm.tile([128, QR], F32, name="mm")
        for j in range(QR // 512):
            nc.tensor.matmul(pps[0:HL, j * 512:(j + 1) * 512], padk,
                             qT[:, r * QR + j * 512: r * QR + (j + 1) * 512],
                             start=True, stop=True)
        nc.scalar.activation(out=e_pad[:, r, :], in_=pps[0:HL, :],
                             func=AF.Exp, scale=SCALE, bias=ln16_sb[0:HL, :])

    while emitted_v < NKT - 1:
        emit_v_tile_fn[0](emitted_v)
        emitted_v += 1

    # (v tiles are emitted inside the block loop above via emit_v_tile_fn)

    # pad rows of x are zero, so v'_pad = bv * sigmoid(bvc) -- no matmul needed.
    # All 16 pad rows are identical; only row 0 is kept (used as a K=1 lhsT).
    sgp = sig3.tile([128, 512], F32, name="sigv")
    nc.scalar.activation(out=sgp[0:1, :], in_=bvc_row.bitcast(F32), func=AF.Sigmoid)
    nc.vector.tensor_tensor(
        out=vt[0:1, NKT - 1, :, 0:VL],
        in0=bv_row.bitcast(F32).rearrange("p (h v) -> p h v", h=HL),
        in1=sgp[0:1, :].rearrange("p (h v) -> p h v", h=HL),
        op=ALU.mult)
    nc.scalar.activation(out=vt[0:1, NKT - 1, :, VL:VL + 1],
                         in_=idr[0:1, 0:HL].unsqueeze(-1),
                         func=AF.Copy, scale=0.0, bias=1.0)
    sig3.release()
    sig2.release()
    ph1.release()
    wpool.release()
    ph13.release()

    # ---------------- phase 4: attention ----------------
    stage = tc.alloc_tile_pool(name="stage", bufs=2)
    expp = tc.alloc_tile_pool(name="expp", bufs=5)
    ctp = tc.alloc_tile_pool(name="ctp", bufs=2)
    outp = tc.alloc_tile_pool(name="outp", bufs=2)
    rcp = tc.alloc_tile_pool(name="rcp", bufs=2)

    def stage_head(qr, h):
        q0 = qr * QR
        qh = stage.tile([QL, QR], F32R, name="qh")
        nc.sync.dma_start(out=qh, in_=qT[h * QL:(h + 1) * QL, q0:q0 + QR])
        kh = stage.tile([QL, S], F32R, name="kh")
        nc.sync.dma_start(out=kh, in_=kT[h * QL:(h + 1) * QL, 0:S])
        ep = stage.tile([1, QR], F32R, name="ep")
        nc.sync.dma_start(out=ep, in_=e_pad[h:h + 1, qr, :])
        return qh, kh, ep

    def head_loop(qr, h, qh, kh, ep, epi=None):
        """scores/exp/ctx over 16 full k tiles; ctx pipelined two k-tiles
        behind the scores so the exp chain never waits on semaphores; the
        pad block lands as a final K=1 rank-1 update. epi (the previous
        head's epilogue, as a generator) is consumed one q-tile per k-tile
        so its PE transposes hide inside the ACT-bound slack."""
        acc = pp_acc.tile([128, QR], F32, name="acc")
        # the pad-block rank-1 update depends on no exp: open the psum
        # accumulation group with it at head start (start=True), freeing the
        # head's tail of everything but the last two ctx flushes
        for j in range(QR // 512):
            nc.tensor.matmul(
                acc[0:VL + 1, j * 512:(j + 1) * 512],
                vt[0:1, NKT - 1, h, :],
                ep[0:1, j * 512:(j + 1) * 512],
                start=True, stop=False)
        pend = []  # (et, t) whose ctx matmuls are not yet emitted
        for t in range(NKT - 1):
            if t >= 5 and epi is not None:
                next(epi, None)
            sc = pp_mm.tile([128, QR], F32, name="mm")
            for j in range(QR // 512):
                nc.tensor.matmul(
                    sc[:, j * 512:(j + 1) * 512],
                    kh[:, t * 128:(t + 1) * 128],
                    qh[:, j * 512:(j + 1) * 512],
                    start=True, stop=True)
            et = expp.tile([128, QR], F32R, name="et")
            nc.scalar.activation(out=et, in_=sc, func=AF.Exp, scale=SCALE)
            pend.append((et, t))
            if len(pend) > 3:
                _emit_ctx(acc, h, *pend.pop(0))
        for p in pend:
            _emit_ctx(acc, h, *p)
        return acc

    def _emit_ctx(acc, h, et, t):
        for j in range(QR // 512):
            nc.tensor.matmul(
                acc[0:VL + 1, j * 512:(j + 1) * 512],
                vt[0:128, t, h, :],
                et[:, j * 512:(j + 1) * 512],
                start=False, stop=(t == NKT - 2))

    def epilogue_copy(acc):
        ct = ctp.tile([VL + 1, QR], F32, name="ct")
        nc.vector.tensor_copy(out=ct, in_=acc[0:VL + 1, :])
        return ct

    def head_epilogue(qr, h, ct, out_sb, dma=False):
        last = h == HL - 1
        for qt in range(QR // 128):
            qsl = slice(qt * 128, (qt + 1) * 128)
            tp = pp_tp.tile([128, 512], F32R, name="tp")
            tpf = tp.bitcast(F32)
            nc.tensor.transpose(
                out=tpf[:, 0:VL + 1],
                in_=ct[:, qsl],
                identity=ident[0:VL + 1, 0:VL + 1])
            rc = rcp.tile([128, 1], F32, name="rc")
            nc.vector.reciprocal(out=rc, in_=tpf[:, VL:VL + 1])
            nc.vector.tensor_scalar_mul(
                out_sb[:, qt, h * VL:(h + 1) * VL], tpf[:, 0:VL], rc)
            yield
        r0 = qr * QR
        yr = y_d[r0:r0 + QR, :].rearrange("(a p) c -> p a c", p=128)
        if h == HL - 2:
            # heads 0..6 are final in cols [0:448): flush them in one DMA
            # during the last head's k-loop so the tail only moves 64 columns
            nc.sync.dma_start(out=yr[:, :, 0:(HL - 1) * VL],
                              in_=out_sb[:, :, 0:(HL - 1) * VL])
        elif last:
            nc.sync.dma_start(out=yr[:, :, (HL - 1) * VL:],
                              in_=out_sb[:, :, (HL - 1) * VL:])
        yield

    # flat (qr, h) stream: every head's epilogue (transposes + divide + DMA)
    # is deferred into the NEXT head's k-loop, including across the qr
    # boundary; only the global last runs inline
    out_sbs = [outp.tile([128, QR // 128, 512], F32, name="out_sb")
               for _ in range(S // QR)]
    prev = None                              # (qr, h, ct) awaiting epilogue
    for qr in range(S // QR):               # 2 q ranges of 1024
        for h in range(HL):
            qh, kh, ep = stage_head(qr, h)
            epi = None
            if prev is not None:
                epi = head_epilogue(prev[0], prev[1], prev[2], out_sbs[prev[0]])
            acc = head_loop(qr, h, qh, kh, ep, epi=epi)
            if epi is not None:
                for _ in epi:
                    pass
            # emit the wide psum->sbuf copy for THIS head immediately (its acc
            # is complete), so the next head's epilogue transposes never wait
            ct = epilogue_copy(acc)
            prev = (qr, h, ct)
    for _ in head_epilogue(prev[0], prev[1], prev[2], out_sbs[prev[0]], dma=True):
        pass

    for p in (rcp, outp, ctp, expp, stage, pp_acc, pp_tp, pp_mm,
              padp, proj, const):
        p.release()


_NC = None


def _get_nc():
    global _NC
    if _NC is None:
        _NC = _build()
    return _NC


def _shard_inputs(inputs):
    x = np.ascontiguousarray(np.asarray(inputs["x"], dtype=np.float32))
    pad = np.zeros((PAD, D), np.float32)
    ident = np.eye(128, dtype=np.float32)
    bdiag = np.repeat(np.eye(8, dtype=np.float32), 16, axis=0)
    in_maps = []
    for c in range(8):
        b, hg = c // 2, c % 2
        qk = slice(hg * 128, (hg + 1) * 128)
        vv = slice(hg * 512, (hg + 1) * 512)
        in_maps.append({
            "x": np.ascontiguousarray(np.concatenate([x[b], pad], axis=0)),
            "idr": ident,
            "bdiag": bdiag,
            "wq": np.ascontiguousarray(inputs["Wq"][:, qk]),
            "wqc": np.ascontiguousarray(inputs["Wqc"][:, qk]),
            "wk": np.ascontiguousarray(inputs["Wk"][:, qk]),
            "wkc": np.ascontiguousarray(inputs["Wkc"][:, qk]),
            "wv": np.ascontiguousarray(inputs["Wv"][:, vv]),
            "wvc": np.ascontiguousarray(inputs["Wvc"][:, vv]),
            "bq": np.ascontiguousarray(inputs["bq"][qk]),
            "bqc": np.ascontiguousarray(inputs["bqc"][qk]),
            "bk": np.ascontiguousarray(inputs["bk"][qk]),
            "bkc": np.ascontiguousarray(inputs["bkc"][qk]),
            "bv": np.ascontiguousarray(inputs["bv"][vv]),
            "bvc": np.ascontiguousarray(inputs["bvc"][vv]),
        })
    return in_maps


def kernel(**inputs) -> np.ndarray:
    nc = _get_nc()
    in_maps = _shard_inputs(inputs)
    res = run_bass_kernel_spmd(nc, in_maps, list(range(8)))
    out = np.empty((B, S, 1024), np.float32)
    for c in range(8):
        b, hg = c // 2, c % 2
        out[b, :, hg * 512:(hg + 1) * 512] = res.results[c]["y"]
    return out


if __name__ == "__main__":
    rng = np.random.default_rng(0)
    d = 1.0 / np.sqrt(D)
    inputs = {
        "x": rng.standard_normal((B, S, D), dtype=np.float32),
        "Wq": rng.standard_normal((D, 256), dtype=np.float32) * d,
        "bq": rng.standard_normal(256).astype(np.float32) * 0.02,
        "Wqc": rng.standard_normal((D, 256), dtype=np.float32) * d,
        "bqc": rng.standard_normal(256).astype(np.float32) * 0.02,
        "Wk": rng.standard_normal((D, 256), dtype=np.float32) * d,
        "bk": rng.standard_normal(256).astype(np.float32) * 0.02,
        "Wkc": rng.standard_normal((D, 256), dtype=np.float32) * d,
        "bkc": rng.standard_normal(256).astype(np.float32) * 0.02,
        "Wv": rng.standard_normal((D, 1024), dtype=np.float32) * d,
        "bv": rng.standard_normal(1024).astype(np.float32) * 0.02,
        "Wvc": rng.standard_normal((D, 1024), dtype=np.float32) * d,
        "bvc": rng.standard_normal(1024).astype(np.float32) * 0.02,
    }
    y = kernel(**inputs)
    print("kernel output", y.shape, y.dtype, float(np.abs(y).max()))



# revision 21
# speedup vs baseline: 1.1880x; 1.1880x over previous
"""Trainium2 Bass kernel for AttentionM (dense transformer block).

Computes, for x [4, 2048, 1024] and q/k/v CSS-gated projections:
    q = (x@Wq+bq)*sigmoid(x@Wqc+bqc)   -> [B, Sp, 16 heads, 16]
    k, v likewise (v 64-wide heads)
    ctx = softmax(q k^T / 8) v          -> [B, S, 1024]
with Sp = S+16 zero-padded rows; pad tokens are bias-only css outputs and
are folded in analytically (no padded x anywhere).

Sharding over 8 NeuronCores: 4-way data parallel over batch x 2-way tensor
parallel over heads (8 heads per core).

Per-core dataflow (low-precision attention; rel err ~1.5e-2 vs 2e-2 budget):
  1. x streams in four 512-token blocks; PE-transposed to feature-major xT.
     k/q projections emit per 512-token chunk: two f32r matmul chains, ACT
     sigmoid, DVE scalar_tensor_tensor writing fp8e4m3 kT8/qT8 directly.
  2. v projections are token-major fp8 (vt8 [128, 8h, 16, 80], col 64 = ones
     denominator, 80-stride so DoubleRow ldweights sees a %16 step); bias via
     K=1 ones-column matmuls; gate mult on DVE writes fp8.
  3. Pad tokens: k_pad = bk*sig(bkc), v_pad = bv*sig(bvc) computed from the
     bias vectors alone (pad x rows are zero). Pad scores for all heads via a
     block-diag fp8 matmul; e_pad = exp(s/8 + ln16) folds the 16 identical
     pad rows; the pad enters each ctx accumulation as an f32r K=1 rank-1
     update (probed: mixes fine with fp8 DoubleRow in one PSUM group).
  4. Attention per (qr in 4 x 512 q, h in 8): scores via DoubleRow fp8
     matmuls (kh8 [8,2,S] x qh8 [8,2,512], two k-tiles' worth of contraction
     per instruction at 0.5 cycles/row); exp on ACT (native Exp -> fp8 out)
     or DVE (Schraudolph: tensor_scalar mult+add -> int8 = e4m3 bit pattern,
     round-half-even convert, zero-mean offset 55.54), greedily balanced by
     projected engine busy-ns; ctx accumulated with one DoubleRow matmul per
     k-tile pair. Softmax denominator rides as vt8 column 64.
  5. Epilogue per (qr,h), deferred one head: PE transpose to token-major,
     DVE reciprocal of the denominator, scale-copy (ACT activation scale=rc
     or DVE tensor_scalar_mul) into out_sb; one output DMA per qr range.
  v tiles and qT8 chunks 1-3 are injected into the early attention emission
  stream so their PE work hides under the exp-bound head loops.
"""

import sys

if "/opt/trn_rl_repo" not in sys.path:
    sys.path.insert(0, "/opt/trn_rl_repo")

import numpy as np

import concourse.bacc as bacc
import concourse.mybir as mybir
import concourse.tile as tile
from concourse.bass_utils import run_bass_kernel_spmd
from concourse.masks import make_identity

F32 = mybir.dt.float32
F32R = mybir.dt.float32r
FP8 = mybir.dt.float8e4
I8 = mybir.dt.int8
AF = mybir.ActivationFunctionType
ALU = mybir.AluOpType
DR = mybir.MatmulPerfMode.DoubleRow

B = 4
S = 2048
D = 1024
DC = 8            # contraction chunks of 128
HL = 8            # heads per core
QL = 16           # q/k head dim
VL = 64           # v head dim
NVT = 16          # real-token v tiles (pad handled analytically)
QR = 512          # q range per attention loop
NQR = S // QR     # 4
NPAIR = 8         # k-tile pairs per loop (16 tiles of 128 = 2048 real k)
SCALE = 1.0 / 8.0
LOG2E = float(np.log2(np.e))
B_SCH = 55.54     # zero-mean Schraudolph offset (RHE convert)
VSTR = 80         # padded v-feature stride (%16 == 0 for DoubleRow ldweights)


def _build(repeat=1):
    nc = bacc.Bacc("TRN2", target_bir_lowering=False, debug=False, num_devices=8)

    x_d = nc.dram_tensor("x", [S, D], F32R, kind="ExternalInput").ap()
    idr_d = nc.dram_tensor("idr", [128, 128], F32R, kind="ExternalInput").ap()
    bdiag_d = nc.dram_tensor("bdiag", [128, 8], F32R, kind="ExternalInput").ap()
    wq_d = nc.dram_tensor("wq", [D, 128], F32R, kind="ExternalInput").ap()
    wqc_d = nc.dram_tensor("wqc", [D, 128], F32R, kind="ExternalInput").ap()
    wk_d = nc.dram_tensor("wk", [D, 128], F32R, kind="ExternalInput").ap()
    wkc_d = nc.dram_tensor("wkc", [D, 128], F32R, kind="ExternalInput").ap()
    wv_d = nc.dram_tensor("wv", [D, 512], F32R, kind="ExternalInput").ap()
    wvc_d = nc.dram_tensor("wvc", [D, 512], F32R, kind="ExternalInput").ap()
    bq_d = nc.dram_tensor("bq", [128], F32, kind="ExternalInput").ap()
    bqc_d = nc.dram_tensor("bqc", [128], F32, kind="ExternalInput").ap()
    bk_d = nc.dram_tensor("bk", [128], F32, kind="ExternalInput").ap()
    bkc_d = nc.dram_tensor("bkc", [128], F32, kind="ExternalInput").ap()
    bv_d = nc.dram_tensor("bv", [512], F32R, kind="ExternalInput").ap()
    bvc_d = nc.dram_tensor("bvc", [512], F32R, kind="ExternalInput").ap()
    y_d = nc.dram_tensor("y", [S, 512], F32, kind="ExternalOutput").ap()

    with tile.TileContext(nc) as tc:
        for _ in range(repeat):
            _emit(nc, tc, x_d, idr_d, bdiag_d, wq_d, wqc_d, wk_d, wkc_d, wv_d,
                  wvc_d, bq_d, bqc_d, bk_d, bkc_d, bv_d, bvc_d, y_d)
    nc.compile()
    return nc


def _emit(nc, tc, x_d, idr_d, bdiag_d, wq_d, wqc_d, wk_d, wkc_d, wv_d,
          wvc_d, bq_d, bqc_d, bk_d, bkc_d, bv_d, bvc_d, y_d):
    # ---------------- pools ----------------
    const = tc.alloc_tile_pool(name="const", bufs=1)
    wpool = tc.alloc_tile_pool(name="wpool", bufs=1)
    ph1 = tc.alloc_tile_pool(name="ph1", bufs=2)
    ph13 = tc.alloc_tile_pool(name="ph13", bufs=1)
    projp = tc.alloc_tile_pool(name="projp", bufs=1)
    sigp = tc.alloc_tile_pool(name="sigp", bufs=2)
    etp = tc.alloc_tile_pool(name="etp", bufs=20)
    stgk = tc.alloc_tile_pool(name="stgk", bufs=2)
    stgq = tc.alloc_tile_pool(name="stgq", bufs=2)
    stge = tc.alloc_tile_pool(name="stge", bufs=3)
    ctp = tc.alloc_tile_pool(name="ctp", bufs=2)
    rcp = tc.alloc_tile_pool(name="rcp", bufs=2)
    outp = tc.alloc_tile_pool(name="outp", bufs=2)
    # PSUM: 3x2 + 1 + 1 = 8 banks
    scp = tc.alloc_tile_pool(name="scp", bufs=3, space="PSUM")
    accp = tc.alloc_tile_pool(name="accp", bufs=1, space="PSUM")
    tpp = tc.alloc_tile_pool(name="tpp", bufs=1, space="PSUM")

    ep0s = {}
    # greedy ACT/DVE balance by projected busy-ns; ACT activation-table
    # switches (Exp vs Sigmoid live in different tables) cost 1283ns each
    busy = {"A": 0.0, "D": 0.0}
    act_tbl = [None]

    def act_table(kind):
        if kind in ("exp", "sigmoid") and act_tbl[0] != kind:
            busy["A"] += 1283
            act_tbl[0] = kind

    def pick(act_cost, dve_cost, act_kind=None):
        extra = 1283 if (act_kind in ("exp", "sigmoid")
                         and act_tbl[0] != act_kind) else 0
        if busy["A"] + act_cost + extra <= busy["D"] + dve_cost:
            busy["A"] += act_cost
            if act_kind:
                act_table(act_kind)
            return "A"
        busy["D"] += dve_cost
        return "D"

    def copy_any(out, in_, free):
        if pick(free * 0.833 + 160, free * 1.042 + 130) == "A":
            nc.scalar.activation(out=out, in_=in_, func=AF.Copy)
        else:
            nc.vector.tensor_copy(out=out, in_=in_)

    # ---------------- constants ----------------
    idr = const.tile([128, 128], F32R, name="idr")
    nc.sync.dma_start(out=idr, in_=idr_d)
    ident = const.tile([128, 128], F32)
    make_identity(nc, ident)

    def ppart(bias_d, dtype=F32):
        t = const.tile([128, 1], dtype, name=f"b_{bias_d.name}")
        nc.sync.dma_start(out=t, in_=bias_d.unsqueeze(-1))
        return t

    # ---------------- long-lived tensors ----------------
    xT_parts = [ph13.tile([128, DC, 512], F32R, name=f"xT{i}") for i in range(4)]

    def xTs(d, c0, csz):
        part = c0 // 512
        lo = c0 - part * 512
        return xT_parts[part][:, d, lo:lo + csz]

    wq = wpool.tile([128, DC, 128], F32R, name="wq")
    wqc = wpool.tile([128, DC, 128], F32R, name="wqc")
    wk = wpool.tile([128, DC, 128], F32R, name="wk")
    wkc = wpool.tile([128, DC, 128], F32R, name="wkc")
    wv = wpool.tile([128, DC, 512], F32R, name="wv")
    wvc = wpool.tile([128, DC, 512], F32R, name="wvc")

    qT8 = projp.tile([128, S], FP8, name="qT8")
    kT8 = projp.tile([128, S], FP8, name="kT8")
    vt8 = projp.tile([128, HL, NVT, VSTR], FP8, name="vt8")
    vt_pad = projp.tile([1, HL, VL + 1], FP8, name="vt_pad")
    e_pad = projp.tile([HL, NQR, QR], FP8, name="e_pad")
    padk8 = projp.tile([128, HL], FP8, name="padk8")
    ln16_sb = projp.tile([128, 1], F32, name="ln16")
    ones_col = const.tile([1, 128], F32R, name="ones_col")

    # ---------------- phase emitters ----------------
    def emit_x_tile(t):
        xt = ph1.tile([128, D], F32R, name="xload")
        nc.sync.dma_start(out=xt, in_=x_d[t * 128:(t + 1) * 128, :])
        return xt

    def emit_tposes(t, xt):
        part = t // 4
        lo = t * 128 - part * 512
        tp = scp.tile([128, 2, 512], F32, name="sc")
        tpf = tp.bitcast(F32R).rearrange("p a b -> p (a b)")
        for d in range(DC):
            nc.tensor.transpose(
                out=tpf[:, d * 128:(d + 1) * 128],
                in_=xt[:, d * 128:(d + 1) * 128],
                identity=idr)
        copy_any(xT_parts[part][:, :, lo:lo + 128],
                 tpf.rearrange("p (b c) -> p b c", b=8), 1024)

    def emit_qk_chunk(wl, wcl, bl, bcl, dest8, c0):
        ps = scp.tile([128, 2, 512], F32, name="sc")
        psf = ps.rearrange("p a b -> p (a b)")
        for d in range(DC):
            nc.tensor.matmul(psf[:, 0:512], wl[:, d, :], xTs(d, c0, 512),
                             start=(d == 0), stop=(d == DC - 1))
        for d in range(DC):
            nc.tensor.matmul(psf[:, 512:1024], wcl[:, d, :], xTs(d, c0, 512),
                             start=(d == 0), stop=(d == DC - 1))
        sig = sigp.tile([128, 512], F32, name="sig")
        busy["A"] += 612
        act_table("sigmoid")
        nc.scalar.activation(out=sig, in_=psf[:, 512:1024],
                             func=AF.Sigmoid, bias=bcl)
        busy["D"] += 658
        nc.vector.scalar_tensor_tensor(
            out=dest8[:, c0:c0 + 512], in0=psf[:, 0:512], scalar=bl,
            in1=sig, op0=ALU.add, op1=ALU.mult)

    def emit_v_tile(t):
        tc0 = t * 128
        ps = scp.tile([128, 2, 512], F32, name="sc")
        psf = ps.rearrange("p a b -> p (a b)")
        for d in range(DC):
            nc.tensor.matmul(psf[:, 0:512], xTs(d, tc0, 128), wv[:, d, :],
                             start=(d == 0), stop=False)
        nc.tensor.matmul(psf[:, 0:512], ones_col, bv_row,
                         start=False, stop=True)
        for d in range(DC):
            nc.tensor.matmul(psf[:, 512:1024], xTs(d, tc0, 128), wvc[:, d, :],
                             start=(d == 0), stop=False)
        nc.tensor.matmul(psf[:, 512:1024], ones_col, bvc_row,
                         start=False, stop=True)
        sg = sigp.tile([128, 512], F32, name="sig")
        busy["A"] += 612
        act_table("sigmoid")
        nc.scalar.activation(out=sg, in_=psf[:, 512:1024], func=AF.Sigmoid)
        busy["D"] += 658
        nc.vector.tensor_tensor(
            out=vt8[:, :, t, 0:VL],
            in0=psf[:, 0:512].rearrange("p (h v) -> p h v", h=HL),
            in1=sg.rearrange("p (h v) -> p h v", h=HL),
            op=ALU.mult)

    def emit_epad_chunk(qr):
        ps = scp.tile([128, 2, 512], F32, name="sc")
        nc.tensor.matmul(ps[0:HL, 0, :], padk8,
                         qT8[:, qr * QR:(qr + 1) * QR], start=True, stop=True)
        busy["A"] += 612
        act_table("exp")
        nc.scalar.activation(out=e_pad[:, qr, :], in_=ps[0:HL, 0, :],
                             func=AF.Exp, scale=SCALE, bias=ln16_sb[0:HL, :])


    # ---------------- P0: x stream, transposes, kT8 ----------------
    bias_done = False
    for blk in range(4):
        for t in range(blk * 4, blk * 4 + 4):
            xt = emit_x_tile(t)
            if not bias_done:
                bq_sb = ppart(bq_d)
                bqc_sb = ppart(bqc_d)
                bk_sb = ppart(bk_d)
                bkc_sb = ppart(bkc_d)
                bv_row = const.tile([1, 512], F32R, name="bv_row")
                nc.sync.dma_start(out=bv_row, in_=bv_d.unsqueeze(0))
                bvc_row = const.tile([1, 512], F32R, name="bvc_row")
                nc.sync.dma_start(out=bvc_row, in_=bvc_d.unsqueeze(0))
                nc.scalar.activation(out=ones_col, in_=idr[0:1, :],
                                     func=AF.Copy, scale=0.0, bias=1.0)
                nc.scalar.activation(out=ln16_sb, in_=idr[:, 0:1], func=AF.Copy,
                                     scale=0.0, bias=float(np.log(16.0)))
                bias_done = True
            emit_tposes(t, xt)
        if blk == 0:
            for w_sb, w_dd in ((wk, wk_d), (wkc, wkc_d), (wq, wq_d), (wqc, wqc_d)):
                nc.sync.dma_start(out=w_sb,
                                  in_=w_dd.rearrange("(a p) c -> p a c", p=128))
        if blk == 1:
            nc.sync.dma_start(out=wv, in_=wv_d.rearrange("(a p) c -> p a c", p=128))
            nc.sync.dma_start(out=wvc, in_=wvc_d.rearrange("(a p) c -> p a c", p=128))
        if blk == 2:
            bdiag = projp.tile([128, HL], F32R, name="bdiag")
            nc.sync.dma_start(out=bdiag, in_=bdiag_d)
        emit_qk_chunk(wk, wkc, bk_sb, bkc_sb, kT8, blk * 512)
        if blk >= 1:
            for t in range((blk - 1) * 4, blk * 4):
                emit_v_tile(t)

    # pad-token constants from biases alone (pad x rows are zero):
    # k_pad = bk*sig(bkc) column, v_pad = bv*sig(bvc) row (+ ones at col 64)
    sigk = sigp.tile([128, 512], F32, name="sig")
    nc.scalar.activation(out=sigk[:, 0:1], in_=bkc_sb, func=AF.Sigmoid)
    kpad = projp.tile([128, 1], F32, name="kpad")
    nc.vector.tensor_tensor(out=kpad, in0=bk_sb, in1=sigk[:, 0:1], op=ALU.mult)
    nc.vector.tensor_scalar(out=padk8, in0=bdiag, scalar1=kpad, scalar2=None,
                            op0=ALU.mult)
    sgp = sigp.tile([128, 512], F32, name="sig")
    nc.scalar.activation(out=sgp[0:1, :], in_=bvc_row.bitcast(F32), func=AF.Sigmoid)
    nc.vector.tensor_tensor(
        out=vt_pad[:, :, 0:VL],
        in0=bv_row.bitcast(F32).rearrange("p (h v) -> p h v", h=HL),
        in1=sgp[0:1, :].rearrange("p (h v) -> p h v", h=HL),
        op=ALU.mult)
    nc.scalar.activation(out=vt_pad[:, :, VL:VL + 1],
                         in_=idr[0:1, 0:HL].unsqueeze(-1),
                         func=AF.Copy, scale=0.0, bias=1.0)
    # denominator ones plane of vt8 via gpsimd memset (SBUF only)
    nc.gpsimd.memset(vt8[:, :, :, VL:VL + 1], 1.0)

    emit_qk_chunk(wq, wqc, bq_sb, bqc_sb, qT8, 0)
    emit_epad_chunk(0)

    # ---------------- injected work for the attention stream ----------------
    inject_q = []                     # closures run one per pair-slot
    for t in range(12, NVT):
        inject_q.append(lambda t=t: emit_v_tile(t))
    for c in range(1, NQR):
        inject_q.append(lambda c=c: emit_qk_chunk(wq, wqc, bq_sb, bqc_sb,
                                                  qT8, c * QR))
        inject_q.append(lambda c=c: emit_epad_chunk(c))

    def inject(n):
        for _ in range(n):
            if inject_q:
                inject_q.pop(0)()

    # ---------------- attention ----------------
    def stage_k(h):
        kh8 = stgk.tile([8, 2, S], FP8, name="kh8")
        nc.sync.dma_start(out=kh8, in_=kT8[16 * h:16 * h + 16, :])
        return kh8

    def stage_qh(qr, h):
        qh8 = stgq.tile([8, 2, QR], FP8, name="qh8")
        sl = slice(qr * QR, (qr + 1) * QR)
        nc.sync.dma_start(out=qh8, in_=qT8[16 * h:16 * h + 16, sl])
        return qh8



    def emit_pair(h, p, kh8, qh8):
        sc = scp.tile([128, 2, 512], F32, name="sc")
        for j in range(2):
            ksl = slice((2 * p + j) * 128, (2 * p + j + 1) * 128)
            nc.tensor.matmul(sc[:, j, :], kh8[:, :, ksl], qh8,
                             start=True, stop=True, perf_mode=DR)
        et8 = etp.tile([128, 2, 512], FP8, name="et8")
        scf = sc.rearrange("p a b -> p (a b)")
        if pick(1080, 1170, act_kind="exp") == "A":
            nc.scalar.activation(out=et8.rearrange("p a b -> p (a b)"),
                                 in_=scf, func=AF.Exp, scale=SCALE)
        else:
            nc.vector.tensor_scalar(
                out=et8.bitcast(I8).rearrange("p a b -> p (a b)"),
                in0=scf, scalar1=LOG2E, scalar2=B_SCH,
                op0=ALU.mult, op1=ALU.add)
        return et8

    def stage_ep(qr, h):
        ep = stge.tile([1, QR], FP8, name="ep")
        nc.sync.dma_start(out=ep, in_=e_pad[h:h + 1, qr, :])
        return ep

    def head_ctx_open(h, ep):
        acc = accp.tile([128, QR], F32, name="acc")
        nc.tensor.matmul(acc[0:VL + 1, :], vt_pad[:, h, :], ep,
                         start=True, stop=False)
        return acc

    def head_loop(qr, h, kh8, qh8, ep, epi, vload):
        acc = None
        pend = []
        for p in range(NPAIR):
            inject(vload)
            if epi is not None:
                next(epi, None)
            et8 = emit_pair(h, p, kh8, qh8)
            pend.append((et8, p))
            if len(pend) > 3:
                if acc is None:
                    acc = head_ctx_open(h, ep)
                _emit_ctx(acc, h, *pend.pop(0))
        for pr in pend:
            if acc is None:
                acc = head_ctx_open(h, ep)
            _emit_ctx(acc, h, *pr)
        return acc

    def _emit_ctx(acc, h, et8, p):
        nc.tensor.matmul(acc[0:VL + 1, :], vt8[:, h, 2 * p:2 * p + 2, 0:VL + 1],
                         et8, start=False, stop=(p == NPAIR - 1), perf_mode=DR)

    def head_epilogue(qr, h, ct, out_sb):
        tp = tpp.tile([128, 4, VL + 1], F32, name="tp")
        for qt in range(4):
            nc.tensor.transpose(
                out=tp[:, qt, :],
                in_=ct[:, qt * 128:(qt + 1) * 128],
                identity=ident[0:VL + 1, 0:VL + 1])
            yield
        rc4 = rcp.tile([128, 4, 1], F32, name="rc")
        nc.vector.reciprocal(out=rc4, in_=tp[:, :, VL:VL + 1])
        busy["D"] += 392
        nc.vector.tensor_tensor(
            out=out_sb[:, :, h * VL:(h + 1) * VL], in0=tp[:, :, 0:VL],
            in1=rc4.to_broadcast([128, 4, VL]), op=ALU.mult)
        if h == HL - 1:
            r0 = qr * QR
            yr = y_d[r0:r0 + QR, :].rearrange("(a p) c -> p a c", p=128)
            nc.sync.dma_start(out=yr, in_=out_sb)
        yield

    # ---- front: scores/exp for ALL of qr0's 8 head-loops, interleaved
    # with the remaining v tiles / qT8 chunks. Heads 0-1 keep their et8 in
    # SBUF; heads 2-7 spill to DRAM (DMA-only cost) and reload for the ctx
    # replay once vt8 is complete. This keeps ACT/DVE fed with exp work
    # through the otherwise PE-bound projection window. ----
    # ---- warmup: scores/exp for (qr0,h0),(qr0,h1) interleaved with v/qk
    # injections; their ctx replays once vt8 is complete ----
    kh8_0 = stage_k(0)
    kh8_1 = stage_k(1)
    qh8_0 = stage_qh(0, 0)
    qh8_1 = stage_qh(0, 1)
    ep_0 = stage_ep(0, 0)
    ep_1 = stage_ep(0, 1)
    stash = {0: [], 1: []}
    for p in range(NPAIR):
        inject(1)
        stash[0].append(emit_pair(0, p, kh8_0, qh8_0))
        inject(1)
        stash[1].append(emit_pair(1, p, kh8_1, qh8_1))
    inject(len(inject_q))
    prev = None                           # (qr, h, ct) awaiting epilogue
    out_sbs = {}
    out_sbs[0] = outp.tile([128, 4, 512], F32, name="out_sb")
    for wh in range(2):
        acc = head_ctx_open(wh, ep_0 if wh == 0 else ep_1)
        for p, et8 in enumerate(stash[wh]):
            _emit_ctx(acc, wh, et8, p)
        ct = ctp.tile([VL + 1, QR], F32, name="ct")
        copy_any(ct, acc[0:VL + 1, :], 512)
        if prev is not None:
            for _ in head_epilogue(prev[0], prev[1], prev[2], out_sbs[0]):
                pass
        prev = (0, wh, ct)
    loops = [(qr, h) for qr in range(NQR) for h in range(HL)][2:]
    kh8_next = stage_k(loops[0][1])
    stq_next = (stage_qh(*loops[0]), stage_ep(*loops[0]))
    for i, (qr, h) in enumerate(loops):
        if h == 0 and qr > 0:
            out_sbs[qr] = outp.tile([128, 4, 512], F32, name="out_sb")
        kh8 = kh8_next
        qh8, ep = stq_next
        if i + 1 < len(loops):
            kh8_next = stage_k(loops[i + 1][1])
            stq_next = (stage_qh(*loops[i + 1]), stage_ep(*loops[i + 1]))
        epi = None
        if prev is not None:
            epi = head_epilogue(prev[0], prev[1], prev[2], out_sbs[prev[0]])
        acc = head_loop(qr, h, kh8, qh8, ep, epi, 0)
        if epi is not None:
            for _ in epi:
                pass
        ct = ctp.tile([VL + 1, QR], F32, name="ct")
        copy_any(ct, acc[0:VL + 1, :], 512)
        prev = (qr, h, ct)
    for _ in head_epilogue(prev[0], prev[1], prev[2], out_sbs[prev[0]]):
        pass

    for p in (tpp, accp, scp, outp, rcp, ctp, stge, stgq, stgk, etp,
              sigp, projp, ph13, ph1, wpool, const):
        p.release()


_NC = None


def _get_nc():
    global _NC
    if _NC is None:
        _NC = _build()
    return _NC


def _shard_inputs(inputs):
    x = np.ascontiguousarray(np.asarray(inputs["x"], dtype=np.float32))
    ident = np.eye(128, dtype=np.float32)
    bdiag = np.repeat(np.eye(8, dtype=np.float32), 16, axis=0)
    in_maps = []
    for c in range(8):
        b, hg = c // 2, c % 2
        qk = slice(hg * 128, (hg + 1) * 128)
        vv = slice(hg * 512, (hg + 1) * 512)
        in_maps.append({
            "x": np.ascontiguousarray(x[b]),
            "idr": ident,
            "bdiag": bdiag,
            "wq": np.ascontiguousarray(inputs["Wq"][:, qk]),
            "wqc": np.ascontiguousarray(inputs["Wqc"][:, qk]),
            "wk": np.ascontiguousarray(inputs["Wk"][:, qk]),
            "wkc": np.ascontiguousarray(inputs["Wkc"][:, qk]),
            "wv": np.ascontiguousarray(inputs["Wv"][:, vv]),
            "wvc": np.ascontiguousarray(inputs["Wvc"][:, vv]),
            "bq": np.ascontiguousarray(inputs["bq"][qk]),
            "bqc": np.ascontiguousarray(inputs["bqc"][qk]),
            "bk": np.ascontiguousarray(inputs["bk"][qk]),
            "bkc": np.ascontiguousarray(inputs["bkc"][qk]),
            "bv": np.ascontiguousarray(inputs["bv"][vv]),
            "bvc": np.ascontiguousarray(inputs["bvc"][vv]),
        })
    return in_maps


def kernel(**inputs) -> np.ndarray:
    nc = _get_nc()
    in_maps = _shard_inputs(inputs)
    res = run_bass_kernel_spmd(nc, in_maps, list(range(8)))
    out = np.empty((B, S, 1024), np.float32)
    for c in range(8):
        b, hg = c // 2, c % 2
        out[b, :, hg * 512:(hg + 1) * 512] = res.results[c]["y"]
    return out


if __name__ == "__main__":
    rng = np.random.default_rng(0)
    d = 1.0 / np.sqrt(D)
    inputs = {
        "x": rng.standard_normal((B, S, D), dtype=np.float32),
        "Wq": rng.standard_normal((D, 256), dtype=np.float32) * d,
        "bq": rng.standard_normal(256).astype(np.float32) * 0.02,
        "Wqc": rng.standard_normal((D, 256), dtype=np.float32) * d,
        "bqc": rng.standard_normal(256).astype(np.float32) * 0.02,
        "Wk": rng.standard_normal((D, 256), dtype=np.float32) * d,
        "bk": rng.standard_normal(256).astype(np.float32) * 0.02,
        "Wkc": rng.standard_normal((D, 256), dtype=np.float32) * d,
        "bkc": rng.standard_normal(256).astype(np.float32) * 0.02,
        "Wv": rng.standard_normal((D, 1024), dtype=np.float32) * d,
        "bv": rng.standard_normal(1024).astype(np.float32) * 0.02,
        "Wvc": rng.standard_normal((D, 1024), dtype=np.float32) * d,
        "bvc": rng.standard_normal(1024).astype(np.float32) * 0.02,
    }
    y = kernel(**inputs)
    print("kernel output", y.shape, y.dtype, float(np.abs(y).max()))


# revision 25
# speedup vs baseline: 1.2115x; 1.0198x over previous
"""Trainium2 Bass kernel for AttentionM (dense transformer block).

Computes, for x [4, 2048, 1024] and q/k/v CSS-gated projections:
    q = (x@Wq+bq)*sigmoid(x@Wqc+bqc)   -> [B, Sp, 16 heads, 16]
    k, v likewise (v 64-wide heads)
    ctx = softmax(q k^T / 8) v          -> [B, S, 1024]
with Sp = S+16 zero-padded rows; pad tokens are bias-only css outputs and
are folded in analytically (no padded x anywhere).

Sharding over 8 NeuronCores: 4-way data parallel over batch x 2-way tensor
parallel over heads (8 heads per core).

Per-core dataflow (low-precision attention, rel err ~1.8e-2 vs 2e-2 budget):
  1. x streams in four 512-token blocks; PE-transposed to feature-major xT
     (f32r, 1.5 cyc/row) and copied to SBUF by whichever of ACT/DVE is
     projected-idler (greedy busy-ns counters steer every flexible op).
  2. k/q projections per 512-token chunk: f32r matmul chains, ACT sigmoid,
     DVE scalar_tensor_tensor writing fp8e4m3 kT8/qT8 directly. v is
     token-major fp8 (vt8 [128, 8h, 16, 80]; 80-stride so DoubleRow
     ldweights sees a %16 interleave step; col 64 = ones denominator via a
     single gpsimd memset); bias via K=1 ones-column matmuls.
  3. Pad tokens: k_pad = bk*sig(bkc), v_pad = bv*sig(bvc) from the bias
     vectors alone (pad x rows are zero). Pad scores for all 8 heads via one
     block-diag fp8 matmul per q-range; e_pad = exp(s/8 + ln16) collapses
     the 16 identical pad rows (fp8, values <= ~22); each ctx accumulation
     opens with an fp8 K=1 rank-1 pad update inside the DoubleRow group.
  4. Attention per (qr in 4 x 512 q, h in 8): scores via DoubleRow fp8
     matmuls at 0.5 cyc/row (kh8/qh8 staged [8,2,*] by one linearizing
     SBUF->SBUF DMA each; interleave lane (p,j) maps head-dim 2p+j on both
     sides). exp on ACT (native Exp -> fp8e4m3 out, exact round-to-nearest)
     or DVE (Schraudolph: one tensor_scalar mult+add -> int8 whose bits ARE
     the e4m3 pattern; round-half-even convert; zero-mean offset 55.54),
     split by projected busy-ns with activation-table switches (Exp vs
     Sigmoid tables) charged 1283ns. ctx accumulates one DoubleRow matmul
     per k-tile pair, trailing exp by 4 pairs.
  5. Epilogue per (qr,h), deferred one head: 4 PE transposes to token-major,
     one DVE reciprocal [128,4,1] of the denominators, one broadcast
     multiply into out_sb; one output DMA per qr range.
  Warmup: scores/exp for (qr0,h0/h1) are emitted interleaved with the
  v-tile / qT8-chunk stream (one injected closure per pair slot) so ACT/DVE
  have exp work during the PE-bound projection window; their ctx replays
  right after. PSUM: 3x[128,2,512] score pairs + acc [128,512] + transpose
  scratch = exactly 8 banks.
"""

import sys

if "/opt/trn_rl_repo" not in sys.path:
    sys.path.insert(0, "/opt/trn_rl_repo")

import numpy as np

import concourse.bacc as bacc
import concourse.mybir as mybir
import concourse.tile as tile
from concourse.bass_utils import run_bass_kernel_spmd
from concourse.masks import make_identity

F32 = mybir.dt.float32
F32R = mybir.dt.float32r
FP8 = mybir.dt.float8e4
I8 = mybir.dt.int8
AF = mybir.ActivationFunctionType
ALU = mybir.AluOpType
DR = mybir.MatmulPerfMode.DoubleRow

B = 4
S = 2048
D = 1024
DC = 8            # contraction chunks of 128
HL = 8            # heads per core
QL = 16           # q/k head dim
VL = 64           # v head dim
NVT = 16          # real-token v tiles (pad handled analytically)
QR = 512          # q range per attention loop
NQR = S // QR     # 4
NPAIR = 8         # k-tile pairs per loop (16 tiles of 128 = 2048 real k)
SCALE = 1.0 / 8.0
LOG2E = float(np.log2(np.e))
B_SCH = 55.54     # zero-mean Schraudolph offset (RHE convert)
VSTR = 80         # padded v-feature stride (%16 == 0 for DoubleRow ldweights)


def _build(repeat=1):
    nc = bacc.Bacc("TRN2", target_bir_lowering=False, debug=False, num_devices=8)

    x_d = nc.dram_tensor("x", [S, D], F32R, kind="ExternalInput").ap()
    idr_d = nc.dram_tensor("idr", [128, 128], F32R, kind="ExternalInput").ap()
    bdiag_d = nc.dram_tensor("bdiag", [128, 8], F32R, kind="ExternalInput").ap()
    wq_d = nc.dram_tensor("wq", [D, 128], F32R, kind="ExternalInput").ap()
    wqc_d = nc.dram_tensor("wqc", [D, 128], F32R, kind="ExternalInput").ap()
    wk_d = nc.dram_tensor("wk", [D, 128], F32R, kind="ExternalInput").ap()
    wkc_d = nc.dram_tensor("wkc", [D, 128], F32R, kind="ExternalInput").ap()
    wv_d = nc.dram_tensor("wv", [D, 512], F32R, kind="ExternalInput").ap()
    wvc_d = nc.dram_tensor("wvc", [D, 512], F32R, kind="ExternalInput").ap()
    bq_d = nc.dram_tensor("bq", [128], F32, kind="ExternalInput").ap()
    bqc_d = nc.dram_tensor("bqc", [128], F32, kind="ExternalInput").ap()
    bk_d = nc.dram_tensor("bk", [128], F32, kind="ExternalInput").ap()
    bkc_d = nc.dram_tensor("bkc", [128], F32, kind="ExternalInput").ap()
    bv_d = nc.dram_tensor("bv", [512], F32R, kind="ExternalInput").ap()
    bvc_d = nc.dram_tensor("bvc", [512], F32R, kind="ExternalInput").ap()
    y_d = nc.dram_tensor("y", [S, 512], F32, kind="ExternalOutput").ap()

    with tile.TileContext(nc) as tc:
        for _ in range(repeat):
            _emit(nc, tc, x_d, idr_d, bdiag_d, wq_d, wqc_d, wk_d, wkc_d, wv_d,
                  wvc_d, bq_d, bqc_d, bk_d, bkc_d, bv_d, bvc_d, y_d)
    nc.compile()
    return nc


def _emit(nc, tc, x_d, idr_d, bdiag_d, wq_d, wqc_d, wk_d, wkc_d, wv_d,
          wvc_d, bq_d, bqc_d, bk_d, bkc_d, bv_d, bvc_d, y_d):
    # ---------------- pools ----------------
    const = tc.alloc_tile_pool(name="const", bufs=1)
    wpool = tc.alloc_tile_pool(name="wpool", bufs=1)
    ph1 = tc.alloc_tile_pool(name="ph1", bufs=2)
    ph13 = tc.alloc_tile_pool(name="ph13", bufs=1)
    projp = tc.alloc_tile_pool(name="projp", bufs=1)
    sigp = tc.alloc_tile_pool(name="sigp", bufs=2)
    etp = tc.alloc_tile_pool(name="etp", bufs=20)
    stgk = tc.alloc_tile_pool(name="stgk", bufs=2)
    stgq = tc.alloc_tile_pool(name="stgq", bufs=2)
    stge = tc.alloc_tile_pool(name="stge", bufs=3)
    ctp = tc.alloc_tile_pool(name="ctp", bufs=2)
    rcp = tc.alloc_tile_pool(name="rcp", bufs=2)
    outp = tc.alloc_tile_pool(name="outp", bufs=2)
    # PSUM: 3x2 + 1 + 1 = 8 banks
    scp = tc.alloc_tile_pool(name="scp", bufs=3, space="PSUM")
    accp = tc.alloc_tile_pool(name="accp", bufs=1, space="PSUM")
    tpp = tc.alloc_tile_pool(name="tpp", bufs=1, space="PSUM")

    ep0s = {}
    # greedy ACT/DVE balance by projected busy-ns; ACT activation-table
    # switches (Exp vs Sigmoid live in different tables) cost 1283ns each
    busy = {"A": 0.0, "D": 0.0}
    act_tbl = [None]

    def act_table(kind):
        if kind in ("exp", "sigmoid") and act_tbl[0] != kind:
            busy["A"] += 1283
            act_tbl[0] = kind

    def pick(act_cost, dve_cost, act_kind=None):
        extra = 1283 if (act_kind in ("exp", "sigmoid")
                         and act_tbl[0] != act_kind) else 0
        if busy["A"] + act_cost + extra <= busy["D"] + dve_cost:
            busy["A"] += act_cost
            if act_kind:
                act_table(act_kind)
            return "A"
        busy["D"] += dve_cost
        return "D"

    def copy_any(out, in_, free):
        if pick(free * 0.833 + 160, free * 1.042 + 130) == "A":
            nc.scalar.activation(out=out, in_=in_, func=AF.Copy)
        else:
            nc.vector.tensor_copy(out=out, in_=in_)

    # ---------------- constants ----------------
    idr = const.tile([128, 128], F32R, name="idr")
    nc.sync.dma_start(out=idr, in_=idr_d)
    ident = const.tile([128, 128], F32)
    make_identity(nc, ident)

    def ppart(bias_d, dtype=F32):
        t = const.tile([128, 1], dtype, name=f"b_{bias_d.name}")
        nc.sync.dma_start(out=t, in_=bias_d.unsqueeze(-1))
        return t

    # ---------------- long-lived tensors ----------------
    xT_parts = [ph13.tile([128, DC, 512], F32R, name=f"xT{i}") for i in range(4)]

    def xTs(d, c0, csz):
        part = c0 // 512
        lo = c0 - part * 512
        return xT_parts[part][:, d, lo:lo + csz]

    wq = wpool.tile([128, DC, 128], F32R, name="wq")
    wqc = wpool.tile([128, DC, 128], F32R, name="wqc")
    wk = wpool.tile([128, DC, 128], F32R, name="wk")
    wkc = wpool.tile([128, DC, 128], F32R, name="wkc")
    wv = wpool.tile([128, DC, 512], F32R, name="wv")
    wvc = wpool.tile([128, DC, 512], F32R, name="wvc")

    qT8 = projp.tile([128, S], FP8, name="qT8")
    kT8 = projp.tile([128, S], FP8, name="kT8")
    vt8 = projp.tile([128, HL, NVT, VSTR], FP8, name="vt8")
    vt_pad = projp.tile([1, HL, VL + 1], FP8, name="vt_pad")
    e_pad = projp.tile([HL, NQR, QR], FP8, name="e_pad")
    padk8 = projp.tile([128, HL], FP8, name="padk8")
    ln16_sb = projp.tile([128, 1], F32, name="ln16")
    ones_col = const.tile([1, 128], F32R, name="ones_col")

    # ---------------- phase emitters ----------------
    def emit_x_tile(t):
        xt = ph1.tile([128, D], F32R, name="xload")
        nc.sync.dma_start(out=xt, in_=x_d[t * 128:(t + 1) * 128, :])
        return xt

    def emit_tposes(t, xt):
        part = t // 4
        lo = t * 128 - part * 512
        tp = scp.tile([128, 2, 512], F32, name="sc")
        tpf = tp.bitcast(F32R).rearrange("p a b -> p (a b)")
        for d in range(DC):
            nc.tensor.transpose(
                out=tpf[:, d * 128:(d + 1) * 128],
                in_=xt[:, d * 128:(d + 1) * 128],
                identity=idr)
        copy_any(xT_parts[part][:, :, lo:lo + 128],
                 tpf.rearrange("p (b c) -> p b c", b=8), 1024)

    def emit_qk_chunk(wl, wcl, bl, bcl, dest8, c0):
        ps = scp.tile([128, 2, 512], F32, name="sc")
        psf = ps.rearrange("p a b -> p (a b)")
        for d in range(DC):
            nc.tensor.matmul(psf[:, 0:512], wl[:, d, :], xTs(d, c0, 512),
                             start=(d == 0), stop=(d == DC - 1))
        for d in range(DC):
            nc.tensor.matmul(psf[:, 512:1024], wcl[:, d, :], xTs(d, c0, 512),
                             start=(d == 0), stop=(d == DC - 1))
        sig = sigp.tile([128, 512], F32, name="sig")
        busy["A"] += 612
        act_table("sigmoid")
        nc.scalar.activation(out=sig, in_=psf[:, 512:1024],
                             func=AF.Sigmoid, bias=bcl)
        busy["D"] += 658
        nc.vector.scalar_tensor_tensor(
            out=dest8[:, c0:c0 + 512], in0=psf[:, 0:512], scalar=bl,
            in1=sig, op0=ALU.add, op1=ALU.mult)

    def emit_v_tile(t):
        tc0 = t * 128
        ps = scp.tile([128, 2, 512], F32, name="sc")
        psf = ps.rearrange("p a b -> p (a b)")
        for d in range(DC):
            nc.tensor.matmul(psf[:, 0:512], xTs(d, tc0, 128), wv[:, d, :],
                             start=(d == 0), stop=False)
        nc.tensor.matmul(psf[:, 0:512], ones_col, bv_row,
                         start=False, stop=True)
        for d in range(DC):
            nc.tensor.matmul(psf[:, 512:1024], xTs(d, tc0, 128), wvc[:, d, :],
                             start=(d == 0), stop=False)
        nc.tensor.matmul(psf[:, 512:1024], ones_col, bvc_row,
                         start=False, stop=True)
        sg = sigp.tile([128, 512], F32, name="sig")
        busy["A"] += 612
        act_table("sigmoid")
        nc.scalar.activation(out=sg, in_=psf[:, 512:1024], func=AF.Sigmoid)
        busy["D"] += 658
        nc.vector.tensor_tensor(
            out=vt8[:, :, t, 0:VL],
            in0=psf[:, 0:512].rearrange("p (h v) -> p h v", h=HL),
            in1=sg.rearrange("p (h v) -> p h v", h=HL),
            op=ALU.mult)

    def emit_epad_chunk(qr):
        ps = scp.tile([128, 2, 512], F32, name="sc")
        nc.tensor.matmul(ps[0:HL, 0, :], padk8,
                         qT8[:, qr * QR:(qr + 1) * QR], start=True, stop=True)
        busy["A"] += 612
        act_table("exp")
        nc.scalar.activation(out=e_pad[:, qr, :], in_=ps[0:HL, 0, :],
                             func=AF.Exp, scale=SCALE, bias=ln16_sb[0:HL, :])


    # ---------------- P0: x stream, transposes, kT8 ----------------
    bias_done = False
    for blk in range(4):
        for t in range(blk * 4, blk * 4 + 4):
            xt = emit_x_tile(t)
            if not bias_done:
                bq_sb = ppart(bq_d)
                bqc_sb = ppart(bqc_d)
                bk_sb = ppart(bk_d)
                bkc_sb = ppart(bkc_d)
                bv_row = const.tile([1, 512], F32R, name="bv_row")
                nc.sync.dma_start(out=bv_row, in_=bv_d.unsqueeze(0))
                bvc_row = const.tile([1, 512], F32R, name="bvc_row")
                nc.sync.dma_start(out=bvc_row, in_=bvc_d.unsqueeze(0))
                nc.scalar.activation(out=ones_col, in_=idr[0:1, :],
                                     func=AF.Copy, scale=0.0, bias=1.0)
                nc.scalar.activation(out=ln16_sb, in_=idr[:, 0:1], func=AF.Copy,
                                     scale=0.0, bias=float(np.log(16.0)))
                bias_done = True
            emit_tposes(t, xt)
        if blk == 0:
            for w_sb, w_dd in ((wk, wk_d), (wkc, wkc_d), (wq, wq_d), (wqc, wqc_d)):
                nc.sync.dma_start(out=w_sb,
                                  in_=w_dd.rearrange("(a p) c -> p a c", p=128))
        if blk == 1:
            nc.sync.dma_start(out=wv, in_=wv_d.rearrange("(a p) c -> p a c", p=128))
            nc.sync.dma_start(out=wvc, in_=wvc_d.rearrange("(a p) c -> p a c", p=128))
        if blk == 2:
            bdiag = projp.tile([128, HL], F32R, name="bdiag")
            nc.sync.dma_start(out=bdiag, in_=bdiag_d)
        emit_qk_chunk(wk, wkc, bk_sb, bkc_sb, kT8, blk * 512)
        if blk >= 2:
            for t in range((blk - 2) * 4, blk * 4 - 4):
                emit_v_tile(t)

    # pad-token constants from biases alone (pad x rows are zero):
    # k_pad = bk*sig(bkc) column, v_pad = bv*sig(bvc) row (+ ones at col 64)
    sigk = sigp.tile([128, 512], F32, name="sig")
    nc.scalar.activation(out=sigk[:, 0:1], in_=bkc_sb, func=AF.Sigmoid)
    kpad = projp.tile([128, 1], F32, name="kpad")
    nc.vector.tensor_tensor(out=kpad, in0=bk_sb, in1=sigk[:, 0:1], op=ALU.mult)
    nc.vector.tensor_scalar(out=padk8, in0=bdiag, scalar1=kpad, scalar2=None,
                            op0=ALU.mult)
    sgp = sigp.tile([128, 512], F32, name="sig")
    nc.scalar.activation(out=sgp[0:1, :], in_=bvc_row.bitcast(F32), func=AF.Sigmoid)
    nc.vector.tensor_tensor(
        out=vt_pad[:, :, 0:VL],
        in0=bv_row.bitcast(F32).rearrange("p (h v) -> p h v", h=HL),
        in1=sgp[0:1, :].rearrange("p (h v) -> p h v", h=HL),
        op=ALU.mult)
    nc.scalar.activation(out=vt_pad[:, :, VL:VL + 1],
                         in_=idr[0:1, 0:HL].unsqueeze(-1),
                         func=AF.Copy, scale=0.0, bias=1.0)
    # denominator ones plane of vt8 via gpsimd memset (SBUF only)
    nc.gpsimd.memset(vt8[:, :, :, VL:VL + 1], 1.0)

    emit_qk_chunk(wq, wqc, bq_sb, bqc_sb, qT8, 0)
    emit_epad_chunk(0)

    # ---------------- injected work for the attention stream ----------------
    inject_q = []                     # closures run one per pair-slot
    for t in range(8, NVT):
        inject_q.append(lambda t=t: emit_v_tile(t))
    for c in range(1, NQR):
        inject_q.append(lambda c=c: emit_qk_chunk(wq, wqc, bq_sb, bqc_sb,
                                                  qT8, c * QR))
        inject_q.append(lambda c=c: emit_epad_chunk(c))

    def inject(n):
        for _ in range(n):
            if inject_q:
                inject_q.pop(0)()

    # ---------------- attention ----------------
    def stage_k(h):
        kh8 = stgk.tile([8, 2, S], FP8, name="kh8")
        nc.sync.dma_start(out=kh8, in_=kT8[16 * h:16 * h + 16, :])
        return kh8

    def stage_qh(qr, h):
        qh8 = stgq.tile([8, 2, QR], FP8, name="qh8")
        sl = slice(qr * QR, (qr + 1) * QR)
        nc.sync.dma_start(out=qh8, in_=qT8[16 * h:16 * h + 16, sl])
        return qh8



    def emit_pair(h, p, kh8, qh8):
        sc = scp.tile([128, 2, 512], F32, name="sc")
        for j in range(2):
            ksl = slice((2 * p + j) * 128, (2 * p + j + 1) * 128)
            nc.tensor.matmul(sc[:, j, :], kh8[:, :, ksl], qh8,
                             start=True, stop=True, perf_mode=DR)
        et8 = etp.tile([128, 2, 512], FP8, name="et8")
        scf = sc.rearrange("p a b -> p (a b)")
        if pick(1080, 1170, act_kind="exp") == "A":
            nc.scalar.activation(out=et8.rearrange("p a b -> p (a b)"),
                                 in_=scf, func=AF.Exp, scale=SCALE)
        else:
            nc.vector.tensor_scalar(
                out=et8.bitcast(I8).rearrange("p a b -> p (a b)"),
                in0=scf, scalar1=LOG2E, scalar2=B_SCH,
                op0=ALU.mult, op1=ALU.add)
        return et8

    def stage_ep(qr, h):
        ep = stge.tile([1, QR], FP8, name="ep")
        nc.sync.dma_start(out=ep, in_=e_pad[h:h + 1, qr, :])
        return ep

    def head_ctx_open(h, ep):
        acc = accp.tile([128, QR], F32, name="acc")
        nc.tensor.matmul(acc[0:VL + 1, :], vt_pad[:, h, :], ep,
                         start=True, stop=False)
        return acc

    def head_loop(qr, h, kh8, qh8, ep, epi, vload):
        acc = None
        pend = []
        for p in range(NPAIR):
            inject(vload)
            if epi is not None:
                next(epi, None)
            et8 = emit_pair(h, p, kh8, qh8)
            pend.append((et8, p))
            if len(pend) > 4:
                if acc is None:
                    acc = head_ctx_open(h, ep)
                _emit_ctx(acc, h, *pend.pop(0))
        for pr in pend:
            if acc is None:
                acc = head_ctx_open(h, ep)
            _emit_ctx(acc, h, *pr)
        return acc

    def _emit_ctx(acc, h, et8, p):
        nc.tensor.matmul(acc[0:VL + 1, :], vt8[:, h, 2 * p:2 * p + 2, 0:VL + 1],
                         et8, start=False, stop=(p == NPAIR - 1), perf_mode=DR)

    def head_epilogue(qr, h, ct, out_sb):
        tp = tpp.tile([128, 4, VL + 1], F32, name="tp")
        for qt in range(4):
            nc.tensor.transpose(
                out=tp[:, qt, :],
                in_=ct[:, qt * 128:(qt + 1) * 128],
                identity=ident[0:VL + 1, 0:VL + 1])
            yield
        rc4 = rcp.tile([128, 4, 1], F32, name="rc")
        nc.vector.reciprocal(out=rc4, in_=tp[:, :, VL:VL + 1])
        busy["D"] += 392
        nc.vector.tensor_tensor(
            out=out_sb[:, :, h * VL:(h + 1) * VL], in0=tp[:, :, 0:VL],
            in1=rc4.to_broadcast([128, 4, VL]), op=ALU.mult)
        if h == HL - 1:
            r0 = qr * QR
            yr = y_d[r0:r0 + QR, :].rearrange("(a p) c -> p a c", p=128)
            nc.sync.dma_start(out=yr, in_=out_sb)
        yield

    # ---- front: scores/exp for ALL of qr0's 8 head-loops, interleaved
    # with the remaining v tiles / qT8 chunks. Heads 0-1 keep their et8 in
    # SBUF; heads 2-7 spill to DRAM (DMA-only cost) and reload for the ctx
    # replay once vt8 is complete. This keeps ACT/DVE fed with exp work
    # through the otherwise PE-bound projection window. ----
    # ---- warmup: scores/exp for (qr0,h0),(qr0,h1) interleaved with v/qk
    # injections; their ctx replays once vt8 is complete ----
    kh8_0 = stage_k(0)
    kh8_1 = stage_k(1)
    qh8_0 = stage_qh(0, 0)
    qh8_1 = stage_qh(0, 1)
    ep_0 = stage_ep(0, 0)
    ep_1 = stage_ep(0, 1)
    stash = {0: [], 1: []}
    for p in range(NPAIR):
        inject(1)
        stash[0].append(emit_pair(0, p, kh8_0, qh8_0))
        inject(1)
        stash[1].append(emit_pair(1, p, kh8_1, qh8_1))
    inject(len(inject_q))
    prev = None                           # (qr, h, ct) awaiting epilogue
    out_sbs = {}
    out_sbs[0] = outp.tile([128, 4, 512], F32, name="out_sb")
    for wh in range(2):
        acc = head_ctx_open(wh, ep_0 if wh == 0 else ep_1)
        for p, et8 in enumerate(stash[wh]):
            _emit_ctx(acc, wh, et8, p)
        ct = ctp.tile([VL + 1, QR], F32, name="ct")
        copy_any(ct, acc[0:VL + 1, :], 512)
        if prev is not None:
            for _ in head_epilogue(prev[0], prev[1], prev[2], out_sbs[0]):
                pass
        prev = (0, wh, ct)
    loops = [(qr, h) for qr in range(NQR) for h in range(HL)][2:]
    kh8_next = stage_k(loops[0][1])
    stq_next = (stage_qh(*loops[0]), stage_ep(*loops[0]))
    for i, (qr, h) in enumerate(loops):
        if h == 0 and qr > 0:
            out_sbs[qr] = outp.tile([128, 4, 512], F32, name="out_sb")
        kh8 = kh8_next
        qh8, ep = stq_next
        if i + 1 < len(loops):
            kh8_next = stage_k(loops[i + 1][1])
            stq_next = (stage_qh(*loops[i + 1]), stage_ep(*loops[i + 1]))
        epi = None
        if prev is not None:
            epi = head_epilogue(prev[0], prev[1], prev[2], out_sbs[prev[0]])
        acc = head_loop(qr, h, kh8, qh8, ep, epi, 0)
        if epi is not None:
            for _ in epi:
                pass
        ct = ctp.tile([VL + 1, QR], F32, name="ct")
        copy_any(ct, acc[0:VL + 1, :], 512)
        prev = (qr, h, ct)
    for _ in head_epilogue(prev[0], prev[1], prev[2], out_sbs[prev[0]]):
        pass

    for p in (tpp, accp, scp, outp, rcp, ctp, stge, stgq, stgk, etp,
              sigp, projp, ph13, ph1, wpool, const):
        p.release()


_NC = None


def _get_nc():
    global _NC
    if _NC is None:
        _NC = _build()
    return _NC


def _shard_inputs(inputs):
    x = np.ascontiguousarray(np.asarray(inputs["x"], dtype=np.float32))
    ident = np.eye(128, dtype=np.float32)
    bdiag = np.repeat(np.eye(8, dtype=np.float32), 16, axis=0)
    in_maps = []
    for c in range(8):
        b, hg = c // 2, c % 2
        qk = slice(hg * 128, (hg + 1) * 128)
        vv = slice(hg * 512, (hg + 1) * 512)
        in_maps.append({
            "x": np.ascontiguousarray(x[b]),
            "idr": ident,
            "bdiag": bdiag,
            "wq": np.ascontiguousarray(inputs["Wq"][:, qk]),
            "wqc": np.ascontiguousarray(inputs["Wqc"][:, qk]),
            "wk": np.ascontiguousarray(inputs["Wk"][:, qk]),
            "wkc": np.ascontiguousarray(inputs["Wkc"][:, qk]),
            "wv": np.ascontiguousarray(inputs["Wv"][:, vv]),
            "wvc": np.ascontiguousarray(inputs["Wvc"][:, vv]),
            "bq": np.ascontiguousarray(inputs["bq"][qk]),
            "bqc": np.ascontiguousarray(inputs["bqc"][qk]),
            "bk": np.ascontiguousarray(inputs["bk"][qk]),
            "bkc": np.ascontiguousarray(inputs["bkc"][qk]),
            "bv": np.ascontiguousarray(inputs["bv"][vv]),
            "bvc": np.ascontiguousarray(inputs["bvc"][vv]),
        })
    return in_maps


def kernel(**inputs) -> np.ndarray:
    nc = _get_nc()
    in_maps = _shard_inputs(inputs)
    res = run_bass_kernel_spmd(nc, in_maps, list(range(8)))
    out = np.empty((B, S, 1024), np.float32)
    for c in range(8):
        b, hg = c // 2, c % 2
        out[b, :, hg * 512:(hg + 1) * 512] = res.results[c]["y"]
    return out


if __name__ == "__main__":
    rng = np.random.default_rng(0)
    d = 1.0 / np.sqrt(D)
    inputs = {
        "x": rng.standard_normal((B, S, D), dtype=np.float32),
        "Wq": rng.standard_normal((D, 256), dtype=np.float32) * d,
        "bq": rng.standard_normal(256).astype(np.float32) * 0.02,
        "Wqc": rng.standard_normal((D, 256), dtype=np.float32) * d,
        "bqc": rng.standard_normal(256).astype(np.float32) * 0.02,
        "Wk": rng.standard_normal((D, 256), dtype=np.float32) * d,
        "bk": rng.standard_normal(256).astype(np.float32) * 0.02,
        "Wkc": rng.standard_normal((D, 256), dtype=np.float32) * d,
        "bkc": rng.standard_normal(256).astype(np.float32) * 0.02,
        "Wv": rng.standard_normal((D, 1024), dtype=np.float32) * d,
        "bv": rng.standard_normal(1024).astype(np.float32) * 0.02,
        "Wvc": rng.standard_normal((D, 1024), dtype=np.float32) * d,
        "bvc": rng.standard_normal(1024).astype(np.float32) * 0.02,
    }
    y = kernel(**inputs)
    print("kernel output", y.shape, y.dtype, float(np.abs(y).max()))


# revision 32
# speedup vs baseline: 1.2218x; 1.0085x over previous
"""Trainium2 Bass kernel for AttentionM (dense transformer block).

Computes, for x [4, 2048, 1024] and q/k/v CSS-gated projections:
    q = (x@Wq+bq)*sigmoid(x@Wqc+bqc)   -> [B, Sp, 16 heads, 16]
    k, v likewise (v 64-wide heads)
    ctx = softmax(q k^T / 8) v          -> [B, S, 1024]
with Sp = S+16 zero-padded rows; pad tokens are bias-only css outputs and
are folded in analytically (no padded x anywhere).

Sharding over 8 NeuronCores: 4-way data parallel over batch x 2-way tensor
parallel over heads (8 heads per core).

Per-core dataflow (low-precision attention, rel err ~1.8e-2 vs 2e-2 budget):
  1. x streams in four 512-token blocks; PE-transposed to feature-major xT
     (f32r, 1.5 cyc/row) and copied to SBUF by whichever of ACT/DVE is
     projected-idler (greedy busy-ns counters steer every flexible op).
  2. k/q projections per 512-token chunk: f32r matmul chains, ACT sigmoid,
     DVE scalar_tensor_tensor writing fp8e4m3 kT8/qT8 directly. v is
     token-major fp8 (vt8 [128, 8h, 16, 80]; 80-stride so DoubleRow
     ldweights sees a %16 interleave step; col 64 = ones denominator via a
     single gpsimd memset); bias via K=1 ones-column matmuls.
  3. Pad tokens: k_pad = bk*sig(bkc), v_pad = bv*sig(bvc) from the bias
     vectors alone (pad x rows are zero). Pad scores for all 8 heads via one
     block-diag fp8 matmul per q-range; e_pad = exp(s/8 + ln16) collapses
     the 16 identical pad rows (fp8, values <= ~22); each ctx accumulation
     opens with an fp8 K=1 rank-1 pad update inside the DoubleRow group.
  4. Attention per (qr in 4 x 512 q, h in 8): scores via DoubleRow fp8
     matmuls at 0.5 cyc/row (kh8/qh8 staged [8,2,*] by one linearizing
     SBUF->SBUF DMA each; interleave lane (p,j) maps head-dim 2p+j on both
     sides). exp on ACT (native Exp -> fp8e4m3 out, exact round-to-nearest)
     or DVE (Schraudolph: one tensor_scalar mult+add -> int8 whose bits ARE
     the e4m3 pattern; round-half-even convert; zero-mean offset 55.54),
     split by projected busy-ns with activation-table switches (Exp vs
     Sigmoid tables) charged 1283ns. ctx accumulates one DoubleRow matmul
     per k-tile pair, trailing exp by 4 pairs.
  5. Epilogue per (qr,h), deferred one head: 4 PE transposes to token-major,
     one DVE reciprocal [128,4,1] of the denominators, one broadcast
     multiply into out_sb; one output DMA per qr range.
  Warmup: scores/exp for (qr0,h0/h1) are emitted interleaved with the
  v-tile / qT8-chunk stream (one injected closure per pair slot) so ACT/DVE
  have exp work during the PE-bound projection window; their ctx replays
  right after. PSUM: 3x[128,2,512] score pairs + acc [128,512] + transpose
  scratch = exactly 8 banks.
"""

import sys

if "/opt/trn_rl_repo" not in sys.path:
    sys.path.insert(0, "/opt/trn_rl_repo")

import numpy as np

import concourse.bacc as bacc
import concourse.mybir as mybir
import concourse.tile as tile
from concourse.bass_utils import run_bass_kernel_spmd
from concourse.masks import make_identity

F32 = mybir.dt.float32
F32R = mybir.dt.float32r
FP8 = mybir.dt.float8e4
I8 = mybir.dt.int8
AF = mybir.ActivationFunctionType
ALU = mybir.AluOpType
DR = mybir.MatmulPerfMode.DoubleRow

B = 4
S = 2048
D = 1024
DC = 8            # contraction chunks of 128
HL = 8            # heads per core
QL = 16           # q/k head dim
VL = 64           # v head dim
NVT = 16          # real-token v tiles (pad handled analytically)
QR = 512          # q range per attention loop
NQR = S // QR     # 4
NPAIR = 8         # k-tile pairs per loop (16 tiles of 128 = 2048 real k)
SCALE = 1.0 / 8.0
LOG2E = float(np.log2(np.e))
B_SCH = 55.54     # zero-mean Schraudolph offset (RHE convert)
VSTR = 80         # padded v-feature stride (%16 == 0 for DoubleRow ldweights)


def _build(repeat=1):
    nc = bacc.Bacc("TRN2", target_bir_lowering=False, debug=False, num_devices=8)

    x_d = nc.dram_tensor("x", [S, D], F32R, kind="ExternalInput").ap()
    idr_d = nc.dram_tensor("idr", [128, 128], F32R, kind="ExternalInput").ap()
    bdiag_d = nc.dram_tensor("bdiag", [128, 8], F32R, kind="ExternalInput").ap()
    wq_d = nc.dram_tensor("wq", [D, 128], F32R, kind="ExternalInput").ap()
    wqc_d = nc.dram_tensor("wqc", [D, 128], F32R, kind="ExternalInput").ap()
    wk_d = nc.dram_tensor("wk", [D, 128], F32R, kind="ExternalInput").ap()
    wkc_d = nc.dram_tensor("wkc", [D, 128], F32R, kind="ExternalInput").ap()
    wv_d = nc.dram_tensor("wv", [D, 512], F32R, kind="ExternalInput").ap()
    wvc_d = nc.dram_tensor("wvc", [D, 512], F32R, kind="ExternalInput").ap()
    bq_d = nc.dram_tensor("bq", [128], F32, kind="ExternalInput").ap()
    bqc_d = nc.dram_tensor("bqc", [128], F32, kind="ExternalInput").ap()
    bk_d = nc.dram_tensor("bk", [128], F32, kind="ExternalInput").ap()
    bkc_d = nc.dram_tensor("bkc", [128], F32, kind="ExternalInput").ap()
    bv_d = nc.dram_tensor("bv", [512], F32R, kind="ExternalInput").ap()
    bvc_d = nc.dram_tensor("bvc", [512], F32R, kind="ExternalInput").ap()
    y_d = nc.dram_tensor("y", [S, 512], F32, kind="ExternalOutput").ap()

    with tile.TileContext(nc) as tc:
        for _ in range(repeat):
            _emit(nc, tc, x_d, idr_d, bdiag_d, wq_d, wqc_d, wk_d, wkc_d, wv_d,
                  wvc_d, bq_d, bqc_d, bk_d, bkc_d, bv_d, bvc_d, y_d)
    nc.compile()
    return nc


def _emit(nc, tc, x_d, idr_d, bdiag_d, wq_d, wqc_d, wk_d, wkc_d, wv_d,
          wvc_d, bq_d, bqc_d, bk_d, bkc_d, bv_d, bvc_d, y_d):
    # ---------------- pools ----------------
    const = tc.alloc_tile_pool(name="const", bufs=1)
    wpool = tc.alloc_tile_pool(name="wpool", bufs=1)
    ph1 = tc.alloc_tile_pool(name="ph1", bufs=3)
    ph13 = tc.alloc_tile_pool(name="ph13", bufs=1)
    projp = tc.alloc_tile_pool(name="projp", bufs=1)
    sigp = tc.alloc_tile_pool(name="sigp", bufs=2)
    etp = tc.alloc_tile_pool(name="etp", bufs=20)
    stgk = tc.alloc_tile_pool(name="stgk", bufs=3)
    stgq = tc.alloc_tile_pool(name="stgq", bufs=2)
    stge = tc.alloc_tile_pool(name="stge", bufs=3)
    ctp = tc.alloc_tile_pool(name="ctp", bufs=2)
    rcp = tc.alloc_tile_pool(name="rcp", bufs=2)
    outp = tc.alloc_tile_pool(name="outp", bufs=2)
    # PSUM: 3x2 + 1 + 1 = 8 banks
    scp = tc.alloc_tile_pool(name="scp", bufs=3, space="PSUM")
    accp = tc.alloc_tile_pool(name="accp", bufs=1, space="PSUM")
    tpp = tc.alloc_tile_pool(name="tpp", bufs=1, space="PSUM")

    ep0s = {}
    # greedy ACT/DVE balance by projected busy-ns; ACT activation-table
    # switches (Exp vs Sigmoid live in different tables) cost 1283ns each
    busy = {"A": 0.0, "D": 0.0}
    act_tbl = [None]

    def act_table(kind):
        if kind in ("exp", "sigmoid") and act_tbl[0] != kind:
            busy["A"] += 1283
            act_tbl[0] = kind

    def pick(act_cost, dve_cost, act_kind=None):
        extra = 1283 if (act_kind in ("exp", "sigmoid")
                         and act_tbl[0] != act_kind) else 0
        if busy["A"] + act_cost + extra <= busy["D"] + dve_cost:
            busy["A"] += act_cost
            if act_kind:
                act_table(act_kind)
            return "A"
        busy["D"] += dve_cost
        return "D"

    def copy_any(out, in_, free):
        if pick(free * 0.833 + 160, free * 1.042 + 130) == "A":
            nc.scalar.activation(out=out, in_=in_, func=AF.Copy)
        else:
            nc.vector.tensor_copy(out=out, in_=in_)

    # ---------------- constants ----------------
    idr = const.tile([128, 128], F32R, name="idr")
    nc.sync.dma_start(out=idr, in_=idr_d)
    ident = const.tile([128, 128], F32)
    make_identity(nc, ident)

    def ppart(bias_d, dtype=F32):
        t = const.tile([128, 1], dtype, name=f"b_{bias_d.name}")
        nc.sync.dma_start(out=t, in_=bias_d.unsqueeze(-1))
        return t

    # ---------------- long-lived tensors ----------------
    xT_parts = [ph13.tile([128, DC, 512], F32R, name=f"xT{i}") for i in range(4)]

    def xTs(d, c0, csz):
        part = c0 // 512
        lo = c0 - part * 512
        return xT_parts[part][:, d, lo:lo + csz]

    wq = wpool.tile([128, DC, 128], F32R, name="wq")
    wqc = wpool.tile([128, DC, 128], F32R, name="wqc")
    wk = wpool.tile([128, DC, 128], F32R, name="wk")
    wkc = wpool.tile([128, DC, 128], F32R, name="wkc")
    wv = wpool.tile([128, DC, 512], F32R, name="wv")
    wvc = wpool.tile([128, DC, 512], F32R, name="wvc")

    qT8 = projp.tile([128, S], FP8, name="qT8")
    kT8 = projp.tile([128, S], FP8, name="kT8")
    vt8 = projp.tile([128, HL, NVT, VSTR], FP8, name="vt8")
    vt_pad = projp.tile([1, HL, VL + 1], FP8, name="vt_pad")
    e_pad = projp.tile([HL, NQR, QR], FP8, name="e_pad")
    padk8 = projp.tile([128, HL], FP8, name="padk8")
    ln16_sb = projp.tile([128, 1], F32, name="ln16")
    ones_col = const.tile([1, 128], F32R, name="ones_col")

    # ---------------- phase emitters ----------------
    def emit_x_tile(t):
        xt = ph1.tile([128, D], F32R, name="xload")
        nc.sync.dma_start(out=xt, in_=x_d[t * 128:(t + 1) * 128, :])
        return xt

    def emit_tposes(t, xt):
        part = t // 4
        lo = t * 128 - part * 512
        tp = scp.tile([128, 2, 512], F32, name="sc")
        tpf = tp.bitcast(F32R).rearrange("p a b -> p (a b)")
        for d in range(DC):
            nc.tensor.transpose(
                out=tpf[:, d * 128:(d + 1) * 128],
                in_=xt[:, d * 128:(d + 1) * 128],
                identity=idr)
        copy_any(xT_parts[part][:, :, lo:lo + 128],
                 tpf.rearrange("p (b c) -> p b c", b=8), 1024)

    def emit_qk_chunk(wl, wcl, bl, bcl, dest8, c0):
        ps = scp.tile([128, 2, 512], F32, name="sc")
        psf = ps.rearrange("p a b -> p (a b)")
        for d in range(DC):
            nc.tensor.matmul(psf[:, 0:512], wl[:, d, :], xTs(d, c0, 512),
                             start=(d == 0), stop=(d == DC - 1))
        for d in range(DC):
            nc.tensor.matmul(psf[:, 512:1024], wcl[:, d, :], xTs(d, c0, 512),
                             start=(d == 0), stop=(d == DC - 1))
        sig = sigp.tile([128, 512], F32, name="sig")
        busy["A"] += 612
        act_table("sigmoid")
        nc.scalar.activation(out=sig, in_=psf[:, 512:1024],
                             func=AF.Sigmoid, bias=bcl)
        busy["D"] += 658
        nc.vector.scalar_tensor_tensor(
            out=dest8[:, c0:c0 + 512], in0=psf[:, 0:512], scalar=bl,
            in1=sig, op0=ALU.add, op1=ALU.mult)

    def emit_v_tile(t):
        tc0 = t * 128
        ps = scp.tile([128, 2, 512], F32, name="sc")
        psf = ps.rearrange("p a b -> p (a b)")
        for d in range(DC):
            nc.tensor.matmul(psf[:, 0:512], xTs(d, tc0, 128), wv[:, d, :],
                             start=(d == 0), stop=False)
        nc.tensor.matmul(psf[:, 0:512], ones_col, bv_row,
                         start=False, stop=True)
        for d in range(DC):
            nc.tensor.matmul(psf[:, 512:1024], xTs(d, tc0, 128), wvc[:, d, :],
                             start=(d == 0), stop=False)
        nc.tensor.matmul(psf[:, 512:1024], ones_col, bvc_row,
                         start=False, stop=True)
        sg = sigp.tile([128, 512], F32, name="sig")
        busy["A"] += 612
        act_table("sigmoid")
        nc.scalar.activation(out=sg, in_=psf[:, 512:1024], func=AF.Sigmoid)
        busy["D"] += 658
        nc.vector.tensor_tensor(
            out=vt8[:, :, t, 0:VL],
            in0=psf[:, 0:512].rearrange("p (h v) -> p h v", h=HL),
            in1=sg.rearrange("p (h v) -> p h v", h=HL),
            op=ALU.mult)

    def emit_epad_chunk(qr):
        ps = scp.tile([128, 2, 512], F32, name="sc")
        nc.tensor.matmul(ps[0:HL, 0, :], padk8,
                         qT8[:, qr * QR:(qr + 1) * QR], start=True, stop=True)
        busy["A"] += 612
        act_table("exp")
        nc.scalar.activation(out=e_pad[:, qr, :], in_=ps[0:HL, 0, :],
                             func=AF.Exp, scale=SCALE, bias=ln16_sb[0:HL, :])


    # ---------------- P0: x stream, transposes, kT8 ----------------
    bias_done = False
    for blk in range(4):
        for t in range(blk * 4, blk * 4 + 4):
            xt = emit_x_tile(t)
            if not bias_done:
                bq_sb = ppart(bq_d)
                bqc_sb = ppart(bqc_d)
                bk_sb = ppart(bk_d)
                bkc_sb = ppart(bkc_d)
                bv_row = const.tile([1, 512], F32R, name="bv_row")
                nc.sync.dma_start(out=bv_row, in_=bv_d.unsqueeze(0))
                bvc_row = const.tile([1, 512], F32R, name="bvc_row")
                nc.sync.dma_start(out=bvc_row, in_=bvc_d.unsqueeze(0))
                nc.scalar.activation(out=ones_col, in_=idr[0:1, :],
                                     func=AF.Copy, scale=0.0, bias=1.0)
                nc.scalar.activation(out=ln16_sb, in_=idr[:, 0:1], func=AF.Copy,
                                     scale=0.0, bias=float(np.log(16.0)))
                bias_done = True
            emit_tposes(t, xt)
        if blk == 0:
            for w_sb, w_dd in ((wk, wk_d), (wkc, wkc_d), (wq, wq_d), (wqc, wqc_d)):
                nc.sync.dma_start(out=w_sb,
                                  in_=w_dd.rearrange("(a p) c -> p a c", p=128))
        if blk == 1:
            nc.sync.dma_start(out=wv, in_=wv_d.rearrange("(a p) c -> p a c", p=128))
            nc.sync.dma_start(out=wvc, in_=wvc_d.rearrange("(a p) c -> p a c", p=128))
        if blk == 2:
            bdiag = projp.tile([128, HL], F32R, name="bdiag")
            nc.sync.dma_start(out=bdiag, in_=bdiag_d)
        emit_qk_chunk(wk, wkc, bk_sb, bkc_sb, kT8, blk * 512)
        if blk >= 2:
            for t in range((blk - 2) * 4, blk * 4 - 4):
                emit_v_tile(t)

    # pad-token constants from biases alone (pad x rows are zero):
    # k_pad = bk*sig(bkc) column, v_pad = bv*sig(bvc) row (+ ones at col 64)
    sigk = sigp.tile([128, 512], F32, name="sig")
    nc.scalar.activation(out=sigk[:, 0:1], in_=bkc_sb, func=AF.Sigmoid)
    kpad = projp.tile([128, 1], F32, name="kpad")
    nc.vector.tensor_tensor(out=kpad, in0=bk_sb, in1=sigk[:, 0:1], op=ALU.mult)
    nc.vector.tensor_scalar(out=padk8, in0=bdiag, scalar1=kpad, scalar2=None,
                            op0=ALU.mult)
    sgp = sigp.tile([128, 512], F32, name="sig")
    nc.scalar.activation(out=sgp[0:1, :], in_=bvc_row.bitcast(F32), func=AF.Sigmoid)
    nc.vector.tensor_tensor(
        out=vt_pad[:, :, 0:VL],
        in0=bv_row.bitcast(F32).rearrange("p (h v) -> p h v", h=HL),
        in1=sgp[0:1, :].rearrange("p (h v) -> p h v", h=HL),
        op=ALU.mult)
    nc.scalar.activation(out=vt_pad[:, :, VL:VL + 1],
                         in_=idr[0:1, 0:HL].unsqueeze(-1),
                         func=AF.Copy, scale=0.0, bias=1.0)
    # denominator ones plane of vt8 via gpsimd memset (SBUF only)
    nc.gpsimd.memset(vt8[:, :, :, VL:VL + 1], 1.0)

    emit_qk_chunk(wq, wqc, bq_sb, bqc_sb, qT8, 0)
    emit_epad_chunk(0)

    # ---------------- injected work for the attention stream ----------------
    inject_q = []                     # closures run one per pair-slot
    for t in range(8, NVT):
        inject_q.append(lambda t=t: emit_v_tile(t))
    for c in range(1, NQR):
        inject_q.append(lambda c=c: emit_qk_chunk(wq, wqc, bq_sb, bqc_sb,
                                                  qT8, c * QR))
        inject_q.append(lambda c=c: emit_epad_chunk(c))

    def inject(n):
        for _ in range(n):
            if inject_q:
                inject_q.pop(0)()

    # ---------------- attention ----------------
    def stage_k(h):
        kh8 = stgk.tile([8, 2, S], FP8, name="kh8")
        nc.sync.dma_start(out=kh8, in_=kT8[16 * h:16 * h + 16, :])
        return kh8

    def stage_qh(qr, h):
        qh8 = stgq.tile([8, 2, QR], FP8, name="qh8")
        sl = slice(qr * QR, (qr + 1) * QR)
        nc.sync.dma_start(out=qh8, in_=qT8[16 * h:16 * h + 16, sl])
        return qh8



    def emit_pair(h, p, kh8, qh8):
        sc = scp.tile([128, 2, 512], F32, name="sc")
        for j in range(2):
            ksl = slice((2 * p + j) * 128, (2 * p + j + 1) * 128)
            nc.tensor.matmul(sc[:, j, :], kh8[:, :, ksl], qh8,
                             start=True, stop=True, perf_mode=DR)
        et8 = etp.tile([128, 2, 512], FP8, name="et8")
        scf = sc.rearrange("p a b -> p (a b)")
        if pick(1080, 1170, act_kind="exp") == "A":
            nc.scalar.activation(out=et8.rearrange("p a b -> p (a b)"),
                                 in_=scf, func=AF.Exp, scale=SCALE)
        else:
            nc.vector.tensor_scalar(
                out=et8.bitcast(I8).rearrange("p a b -> p (a b)"),
                in0=scf, scalar1=LOG2E, scalar2=B_SCH,
                op0=ALU.mult, op1=ALU.add)
        return et8

    def stage_ep(qr, h):
        ep = stge.tile([1, QR], FP8, name="ep")
        nc.sync.dma_start(out=ep, in_=e_pad[h:h + 1, qr, :])
        return ep

    def head_ctx_open(h, ep):
        acc = accp.tile([128, QR], F32, name="acc")
        nc.tensor.matmul(acc[0:VL + 1, :], vt_pad[:, h, :], ep,
                         start=True, stop=False)
        return acc

    def head_loop(qr, h, kh8, qh8, ep, epi, vload):
        acc = None
        pend = []
        for p in range(NPAIR):
            inject(vload)
            if epi is not None:
                next(epi, None)
            et8 = emit_pair(h, p, kh8, qh8)
            pend.append((et8, p))
            if len(pend) > 4:
                if acc is None:
                    acc = head_ctx_open(h, ep)
                _emit_ctx(acc, h, *pend.pop(0))
        for pr in pend:
            if acc is None:
                acc = head_ctx_open(h, ep)
            _emit_ctx(acc, h, *pr)
        return acc

    def _emit_ctx(acc, h, et8, p):
        nc.tensor.matmul(acc[0:VL + 1, :], vt8[:, h, 2 * p:2 * p + 2, 0:VL + 1],
                         et8, start=False, stop=(p == NPAIR - 1), perf_mode=DR)

    def head_epilogue(qr, h, ct, out_sb):
        tp = tpp.tile([128, 4, VL + 1], F32, name="tp")
        for qt in range(4):
            nc.tensor.transpose(
                out=tp[:, qt, :],
                in_=ct[:, qt * 128:(qt + 1) * 128],
                identity=ident[0:VL + 1, 0:VL + 1])
            yield
        rc4 = rcp.tile([128, 4, 1], F32, name="rc")
        nc.vector.reciprocal(out=rc4, in_=tp[:, :, VL:VL + 1])
        busy["D"] += 392
        nc.vector.tensor_tensor(
            out=out_sb[:, :, h * VL:(h + 1) * VL], in0=tp[:, :, 0:VL],
            in1=rc4.to_broadcast([128, 4, VL]), op=ALU.mult)
        if h == HL - 1:
            r0 = qr * QR
            yr = y_d[r0:r0 + QR, :].rearrange("(a p) c -> p a c", p=128)
            nc.sync.dma_start(out=yr, in_=out_sb)
        yield

    # ---- front: scores/exp for ALL of qr0's 8 head-loops, interleaved
    # with the remaining v tiles / qT8 chunks. Heads 0-1 keep their et8 in
    # SBUF; heads 2-7 spill to DRAM (DMA-only cost) and reload for the ctx
    # replay once vt8 is complete. This keeps ACT/DVE fed with exp work
    # through the otherwise PE-bound projection window. ----
    # ---- warmup: scores/exp for (qr0,h0),(qr0,h1) interleaved with v/qk
    # injections; their ctx replays once vt8 is complete ----
    kh8_0 = stage_k(0)
    kh8_1 = stage_k(1)
    qh8_0 = stage_qh(0, 0)
    qh8_1 = stage_qh(0, 1)
    ep_0 = stage_ep(0, 0)
    ep_1 = stage_ep(0, 1)
    stash = {0: [], 1: []}
    for p in range(NPAIR):
        inject(1)
        stash[0].append(emit_pair(0, p, kh8_0, qh8_0))
        inject(1)
        stash[1].append(emit_pair(1, p, kh8_1, qh8_1))
    inject(len(inject_q))
    prev = None                           # (qr, h, ct) awaiting epilogue
    out_sbs = {}
    out_sbs[0] = outp.tile([128, 4, 512], F32, name="out_sb")
    for wh in range(2):
        acc = head_ctx_open(wh, ep_0 if wh == 0 else ep_1)
        for p, et8 in enumerate(stash[wh]):
            _emit_ctx(acc, wh, et8, p)
        ct = ctp.tile([VL + 1, QR], F32, name="ct")
        copy_any(ct, acc[0:VL + 1, :], 512)
        if prev is not None:
            for _ in head_epilogue(prev[0], prev[1], prev[2], out_sbs[0]):
                pass
        prev = (0, wh, ct)
    loops = [(qr, h) for qr in range(NQR) for h in range(HL)][2:]
    kh8_next = stage_k(loops[0][1])
    stq_next = (stage_qh(*loops[0]), stage_ep(*loops[0]))
    for i, (qr, h) in enumerate(loops):
        if h == 0 and qr > 0:
            out_sbs[qr] = outp.tile([128, 4, 512], F32, name="out_sb")
        kh8 = kh8_next
        qh8, ep = stq_next
        if i + 1 < len(loops):
            kh8_next = stage_k(loops[i + 1][1])
            stq_next = (stage_qh(*loops[i + 1]), stage_ep(*loops[i + 1]))
        epi = None
        if prev is not None:
            epi = head_epilogue(prev[0], prev[1], prev[2], out_sbs[prev[0]])
        acc = head_loop(qr, h, kh8, qh8, ep, epi, 0)
        if epi is not None:
            for _ in epi:
                pass
        ct = ctp.tile([VL + 1, QR], F32, name="ct")
        copy_any(ct, acc[0:VL + 1, :], 512)
        prev = (qr, h, ct)
    for _ in head_epilogue(prev[0], prev[1], prev[2], out_sbs[prev[0]]):
        pass

    for p in (tpp, accp, scp, outp, rcp, ctp, stge, stgq, stgk, etp,
              sigp, projp, ph13, ph1, wpool, const):
        p.release()


_NC = None


def _get_nc():
    global _NC
    if _NC is None:
        _NC = _build()
    return _NC


def _shard_inputs(inputs):
    x = np.ascontiguousarray(np.asarray(inputs["x"], dtype=np.float32))
    ident = np.eye(128, dtype=np.float32)
    bdiag = np.repeat(np.eye(8, dtype=np.float32), 16, axis=0)
    in_maps = []
    for c in range(8):
        b, hg = c // 2, c % 2
        qk = slice(hg * 128, (hg + 1) * 128)
        vv = slice(hg * 512, (hg + 1) * 512)
        in_maps.append({
            "x": np.ascontiguousarray(x[b]),
            "idr": ident,
            "bdiag": bdiag,
            "wq": np.ascontiguousarray(inputs["Wq"][:, qk]),
            "wqc": np.ascontiguousarray(inputs["Wqc"][:, qk]),
            "wk": np.ascontiguousarray(inputs["Wk"][:, qk]),
            "wkc": np.ascontiguousarray(inputs["Wkc"][:, qk]),
            "wv": np.ascontiguousarray(inputs["Wv"][:, vv]),
            "wvc": np.ascontiguousarray(inputs["Wvc"][:, vv]),
            "bq": np.ascontiguousarray(inputs["bq"][qk]),
            "bqc": np.ascontiguousarray(inputs["bqc"][qk]),
            "bk": np.ascontiguousarray(inputs["bk"][qk]),
            "bkc": np.ascontiguousarray(inputs["bkc"][qk]),
            "bv": np.ascontiguousarray(inputs["bv"][vv]),
            "bvc": np.ascontiguousarray(inputs["bvc"][vv]),
        })
    return in_maps


def kernel(**inputs) -> np.ndarray:
    nc = _get_nc()
    in_maps = _shard_inputs(inputs)
    res = run_bass_kernel_spmd(nc, in_maps, list(range(8)))
    out = np.empty((B, S, 1024), np.float32)
    for c in range(8):
        b, hg = c // 2, c % 2
        out[b, :, hg * 512:(hg + 1) * 512] = res.results[c]["y"]
    return out


if __name__ == "__main__":
    rng = np.random.default_rng(0)
    d = 1.0 / np.sqrt(D)
    inputs = {
        "x": rng.standard_normal((B, S, D), dtype=np.float32),
        "Wq": rng.standard_normal((D, 256), dtype=np.float32) * d,
        "bq": rng.standard_normal(256).astype(np.float32) * 0.02,
        "Wqc": rng.standard_normal((D, 256), dtype=np.float32) * d,
        "bqc": rng.standard_normal(256).astype(np.float32) * 0.02,
        "Wk": rng.standard_normal((D, 256), dtype=np.float32) * d,
        "bk": rng.standard_normal(256).astype(np.float32) * 0.02,
        "Wkc": rng.standard_normal((D, 256), dtype=np.float32) * d,
        "bkc": rng.standard_normal(256).astype(np.float32) * 0.02,
        "Wv": rng.standard_normal((D, 1024), dtype=np.float32) * d,
        "bv": rng.standard_normal(1024).astype(np.float32) * 0.02,
        "Wvc": rng.standard_normal((D, 1024), dtype=np.float32) * d,
        "bvc": rng.standard_normal(1024).astype(np.float32) * 0.02,
    }
    y = kernel(**inputs)
    print("kernel output", y.shape, y.dtype, float(np.abs(y).max()))


# revision 38
# speedup vs baseline: 1.2590x; 1.0305x over previous
"""Trainium2 Bass kernel for AttentionM (dense transformer block).

Computes, for x [4, 2048, 1024] and q/k/v CSS-gated projections:
    q = (x@Wq+bq)*sigmoid(x@Wqc+bqc)   -> [B, Sp, 16 heads, 16]
    k, v likewise (v 64-wide heads)
    ctx = softmax(q k^T / 8) v          -> [B, S, 1024]
with Sp = S+16 zero-padded rows; pad tokens are bias-only css outputs and
are folded in analytically (no padded x anywhere).

Sharding over 8 NeuronCores: 4-way data parallel over batch x 2-way tensor
parallel over heads (8 heads per core).

Per-core dataflow (low-precision attention, rel err ~1.8e-2 vs 2e-2 budget):
  1. x streams in four 512-token blocks; PE-transposed to feature-major xT
     (f32r, 1.5 cyc/row) and copied to SBUF by whichever of ACT/DVE is
     projected-idler (greedy busy-ns counters steer every flexible op).
  2. k/q projections per 512-token chunk: f32r matmul chains, ACT sigmoid,
     DVE scalar_tensor_tensor writing fp8e4m3 kT8/qT8 directly. v is
     token-major fp8 (vt8 [128, 8h, 16, 80]; 80-stride so DoubleRow
     ldweights sees a %16 interleave step; col 64 = ones denominator via a
     single gpsimd memset); bias via K=1 ones-column matmuls.
  3. Pad tokens: k_pad = bk*sig(bkc), v_pad = bv*sig(bvc) from the bias
     vectors alone (pad x rows are zero). Pad scores for all 8 heads via one
     block-diag fp8 matmul per q-range; e_pad = exp(s/8 + ln16) collapses
     the 16 identical pad rows (fp8, values <= ~22); each ctx accumulation
     opens with an fp8 K=1 rank-1 pad update inside the DoubleRow group.
  4. Attention per (qr in 4 x 512 q, h in 8): scores via DoubleRow fp8
     matmuls at 0.5 cyc/row (kh8/qh8 staged [8,2,*] by one linearizing
     SBUF->SBUF DMA each; interleave lane (p,j) maps head-dim 2p+j on both
     sides). exp on ACT (native Exp -> fp8e4m3 out, exact round-to-nearest)
     or DVE (Schraudolph: one tensor_scalar mult+add -> int8 whose bits ARE
     the e4m3 pattern; round-half-even convert; zero-mean offset 55.54),
     split by projected busy-ns with activation-table switches (Exp vs
     Sigmoid tables) charged 1283ns. ctx accumulates one DoubleRow matmul
     per k-tile pair, trailing exp by 4 pairs.
  5. Epilogue per (qr,h), deferred one head: 4 PE transposes to token-major,
     one DVE reciprocal [128,4,1] of the denominators, one broadcast
     multiply into out_sb; one output DMA per qr range.
  Warmup: scores/exp for (qr0,h0/h1) are emitted interleaved with the
  v-tile / qT8-chunk stream (one injected closure per pair slot) so ACT/DVE
  have exp work during the PE-bound projection window; their ctx replays
  right after. PSUM: 3x[128,2,512] score pairs + acc [128,512] + transpose
  scratch = exactly 8 banks.
"""

import sys

if "/opt/trn_rl_repo" not in sys.path:
    sys.path.insert(0, "/opt/trn_rl_repo")

import numpy as np

import concourse.bacc as bacc
import concourse.mybir as mybir
import concourse.tile as tile
from concourse.bass_utils import run_bass_kernel_spmd
from concourse.masks import make_identity

F32 = mybir.dt.float32
F32R = mybir.dt.float32r
FP8 = mybir.dt.float8e4
I8 = mybir.dt.int8
AF = mybir.ActivationFunctionType
ALU = mybir.AluOpType
DR = mybir.MatmulPerfMode.DoubleRow

B = 4
S = 2048
D = 1024
DC = 8            # contraction chunks of 128
HL = 8            # heads per core
QL = 16           # q/k head dim
VL = 64           # v head dim
NVT = 16          # real-token v tiles (pad handled analytically)
QR = 512          # q range per attention loop
NQR = S // QR     # 4
NPAIR = 8         # k-tile pairs per loop (16 tiles of 128 = 2048 real k)
SCALE = 1.0 / 8.0
LOG2E = float(np.log2(np.e))
B_SCH = 55.54     # zero-mean Schraudolph offset (RHE convert)
VSTR = 80         # padded v-feature stride (%16 == 0 for DoubleRow ldweights)


def _build(repeat=1):
    nc = bacc.Bacc("TRN2", target_bir_lowering=False, debug=False, num_devices=8)

    x_d = nc.dram_tensor("x", [S, D], F32R, kind="ExternalInput").ap()
    idr_d = nc.dram_tensor("idr", [128, 128], F32R, kind="ExternalInput").ap()
    bdiag_d = nc.dram_tensor("bdiag", [128, 8], F32R, kind="ExternalInput").ap()
    wq_d = nc.dram_tensor("wq", [D, 128], F32R, kind="ExternalInput").ap()
    wqc_d = nc.dram_tensor("wqc", [D, 128], F32R, kind="ExternalInput").ap()
    wk_d = nc.dram_tensor("wk", [D, 128], F32R, kind="ExternalInput").ap()
    wkc_d = nc.dram_tensor("wkc", [D, 128], F32R, kind="ExternalInput").ap()
    wv_d = nc.dram_tensor("wv", [D, 512], F32R, kind="ExternalInput").ap()
    wvc_d = nc.dram_tensor("wvc", [D, 512], F32R, kind="ExternalInput").ap()
    bq_d = nc.dram_tensor("bq", [128], F32, kind="ExternalInput").ap()
    bqc_d = nc.dram_tensor("bqc", [128], F32, kind="ExternalInput").ap()
    bk_d = nc.dram_tensor("bk", [128], F32, kind="ExternalInput").ap()
    bkc_d = nc.dram_tensor("bkc", [128], F32, kind="ExternalInput").ap()
    bv_d = nc.dram_tensor("bv", [512], F32R, kind="ExternalInput").ap()
    bvc_d = nc.dram_tensor("bvc", [512], F32R, kind="ExternalInput").ap()
    y_d = nc.dram_tensor("y", [S, 512], F32, kind="ExternalOutput").ap()

    with tile.TileContext(nc) as tc:
        for _ in range(repeat):
            _emit(nc, tc, x_d, idr_d, bdiag_d, wq_d, wqc_d, wk_d, wkc_d, wv_d,
                  wvc_d, bq_d, bqc_d, bk_d, bkc_d, bv_d, bvc_d, y_d)
    nc.compile()
    return nc


def _emit(nc, tc, x_d, idr_d, bdiag_d, wq_d, wqc_d, wk_d, wkc_d, wv_d,
          wvc_d, bq_d, bqc_d, bk_d, bkc_d, bv_d, bvc_d, y_d):
    # ---------------- pools ----------------
    const = tc.alloc_tile_pool(name="const", bufs=1)
    wpool = tc.alloc_tile_pool(name="wpool", bufs=1)
    ph1 = tc.alloc_tile_pool(name="ph1", bufs=3)
    ph13 = tc.alloc_tile_pool(name="ph13", bufs=1)
    projp = tc.alloc_tile_pool(name="projp", bufs=1)
    sigp = tc.alloc_tile_pool(name="sigp", bufs=2)
    etp = tc.alloc_tile_pool(name="etp", bufs=20)
    stgk = tc.alloc_tile_pool(name="stgk", bufs=3)
    stgq = tc.alloc_tile_pool(name="stgq", bufs=2)
    stge = tc.alloc_tile_pool(name="stge", bufs=3)
    ctp = tc.alloc_tile_pool(name="ctp", bufs=2)
    rcp = tc.alloc_tile_pool(name="rcp", bufs=2)
    outp = tc.alloc_tile_pool(name="outp", bufs=2)
    # PSUM: 3x2 + 1 + 1 = 8 banks
    scp = tc.alloc_tile_pool(name="scp", bufs=3, space="PSUM")
    accp = tc.alloc_tile_pool(name="accp", bufs=1, space="PSUM")
    tpp = tc.alloc_tile_pool(name="tpp", bufs=1, space="PSUM")

    ep0s = {}
    # greedy ACT/DVE balance by projected busy-ns; ACT activation-table
    # switches (Exp vs Sigmoid live in different tables) cost 1283ns each
    busy = {"A": 0.0, "D": 0.0}
    act_tbl = [None]

    def act_table(kind):
        if kind in ("exp", "sigmoid") and act_tbl[0] != kind:
            busy["A"] += 1283
            act_tbl[0] = kind

    def pick(act_cost, dve_cost, act_kind=None):
        extra = 1283 if (act_kind in ("exp", "sigmoid")
                         and act_tbl[0] != act_kind) else 0
        if busy["A"] + act_cost + extra <= busy["D"] + dve_cost:
            busy["A"] += act_cost
            if act_kind:
                act_table(act_kind)
            return "A"
        busy["D"] += dve_cost
        return "D"

    def copy_any(out, in_, free):
        if pick(free * 0.833 + 160, free * 1.042 + 200) == "A":
            nc.scalar.activation(out=out, in_=in_, func=AF.Copy)
        else:
            nc.vector.tensor_copy(out=out, in_=in_)

    # ---------------- constants ----------------
    idr = const.tile([128, 128], F32R, name="idr")
    nc.sync.dma_start(out=idr, in_=idr_d)
    ident = const.tile([128, 128], F32)
    make_identity(nc, ident)

    def ppart(bias_d, dtype=F32):
        t = const.tile([128, 1], dtype, name=f"b_{bias_d.name}")
        nc.sync.dma_start(out=t, in_=bias_d.unsqueeze(-1))
        return t

    # ---------------- long-lived tensors ----------------
    xT_parts = [ph13.tile([128, DC, 512], F32R, name=f"xT{i}") for i in range(4)]

    def xTs(d, c0, csz):
        part = c0 // 512
        lo = c0 - part * 512
        return xT_parts[part][:, d, lo:lo + csz]

    wq = wpool.tile([128, DC, 128], F32R, name="wq")
    wqc = wpool.tile([128, DC, 128], F32R, name="wqc")
    wk = wpool.tile([128, DC, 128], F32R, name="wk")
    wkc = wpool.tile([128, DC, 128], F32R, name="wkc")
    wv = wpool.tile([128, DC, 512], F32R, name="wv")
    wvc = wpool.tile([128, DC, 512], F32R, name="wvc")

    qT8 = projp.tile([128, S], FP8, name="qT8")
    kT8 = projp.tile([128, S], FP8, name="kT8")
    vt8 = projp.tile([128, HL, NVT, VSTR], FP8, name="vt8")
    vt_pad = projp.tile([1, HL, VL + 1], FP8, name="vt_pad")
    e_pad = projp.tile([HL, NQR, QR], FP8, name="e_pad")
    padk8 = projp.tile([128, HL], FP8, name="padk8")
    ln16_sb = projp.tile([128, 1], F32, name="ln16")
    ones_col = const.tile([1, 128], F32R, name="ones_col")

    # ---------------- phase emitters ----------------
    def emit_x_tile(t):
        xt = ph1.tile([128, D], F32R, name="xload")
        nc.sync.dma_start(out=xt, in_=x_d[t * 128:(t + 1) * 128, :])
        return xt

    def emit_tposes(t, xt):
        part = t // 4
        lo = t * 128 - part * 512
        tp = scp.tile([128, 2, 512], F32, name="sc")
        tpf = tp.bitcast(F32R).rearrange("p a b -> p (a b)")
        for d in range(DC):
            nc.tensor.transpose(
                out=tpf[:, d * 128:(d + 1) * 128],
                in_=xt[:, d * 128:(d + 1) * 128],
                identity=idr)
        copy_any(xT_parts[part][:, :, lo:lo + 128],
                 tpf.rearrange("p (b c) -> p b c", b=8), 1024)

    def emit_qk_chunk(wl, wcl, bl, bcl, dest8, c0):
        ps = scp.tile([128, 2, 512], F32, name="sc")
        psf = ps.rearrange("p a b -> p (a b)")
        for d in range(DC):
            nc.tensor.matmul(psf[:, 0:512], wl[:, d, :], xTs(d, c0, 512),
                             start=(d == 0), stop=(d == DC - 1))
        for d in range(DC):
            nc.tensor.matmul(psf[:, 512:1024], wcl[:, d, :], xTs(d, c0, 512),
                             start=(d == 0), stop=(d == DC - 1))
        sig = sigp.tile([128, 512], F32, name="sig")
        busy["A"] += 612
        act_table("sigmoid")
        nc.scalar.activation(out=sig, in_=psf[:, 512:1024],
                             func=AF.Sigmoid, bias=bcl)
        busy["D"] += 658
        nc.vector.scalar_tensor_tensor(
            out=dest8[:, c0:c0 + 512], in0=psf[:, 0:512], scalar=bl,
            in1=sig, op0=ALU.add, op1=ALU.mult)

    def emit_v_tile(t):
        tc0 = t * 128
        ps = scp.tile([128, 2, 512], F32, name="sc")
        psf = ps.rearrange("p a b -> p (a b)")
        for d in range(DC):
            nc.tensor.matmul(psf[:, 0:512], xTs(d, tc0, 128), wv[:, d, :],
                             start=(d == 0), stop=False)
        nc.tensor.matmul(psf[:, 0:512], ones_col, bv_row,
                         start=False, stop=True)
        for d in range(DC):
            nc.tensor.matmul(psf[:, 512:1024], xTs(d, tc0, 128), wvc[:, d, :],
                             start=(d == 0), stop=False)
        nc.tensor.matmul(psf[:, 512:1024], ones_col, bvc_row,
                         start=False, stop=True)
        sg = sigp.tile([128, 512], F32, name="sig")
        busy["A"] += 612
        act_table("sigmoid")
        nc.scalar.activation(out=sg, in_=psf[:, 512:1024], func=AF.Sigmoid)
        busy["D"] += 658
        nc.vector.tensor_tensor(
            out=vt8[:, :, t, 0:VL],
            in0=psf[:, 0:512].rearrange("p (h v) -> p h v", h=HL),
            in1=sg.rearrange("p (h v) -> p h v", h=HL),
            op=ALU.mult)

    def emit_epad_chunk(qr):
        ps = scp.tile([128, 2, 512], F32, name="sc")
        nc.tensor.matmul(ps[0:HL, 0, :], padk8,
                         qT8[:, qr * QR:(qr + 1) * QR], start=True, stop=True)
        busy["A"] += 612
        act_table("exp")
        nc.scalar.activation(out=e_pad[:, qr, :], in_=ps[0:HL, 0, :],
                             func=AF.Exp, scale=SCALE, bias=ln16_sb[0:HL, :])


    # ---------------- P0: x stream, transposes, kT8 ----------------
    bias_done = False
    for blk in range(4):
        for t in range(blk * 4, blk * 4 + 4):
            xt = emit_x_tile(t)
            if not bias_done:
                bq_sb = ppart(bq_d)
                bqc_sb = ppart(bqc_d)
                bk_sb = ppart(bk_d)
                bkc_sb = ppart(bkc_d)
                bv_row = const.tile([1, 512], F32R, name="bv_row")
                nc.sync.dma_start(out=bv_row, in_=bv_d.unsqueeze(0))
                bvc_row = const.tile([1, 512], F32R, name="bvc_row")
                nc.sync.dma_start(out=bvc_row, in_=bvc_d.unsqueeze(0))
                nc.scalar.activation(out=ones_col, in_=idr[0:1, :],
                                     func=AF.Copy, scale=0.0, bias=1.0)
                nc.scalar.activation(out=ln16_sb, in_=idr[:, 0:1], func=AF.Copy,
                                     scale=0.0, bias=float(np.log(16.0)))
                bias_done = True
            emit_tposes(t, xt)
        if blk == 0:
            for w_sb, w_dd in ((wk, wk_d), (wkc, wkc_d), (wq, wq_d), (wqc, wqc_d)):
                nc.sync.dma_start(out=w_sb,
                                  in_=w_dd.rearrange("(a p) c -> p a c", p=128))
        if blk == 2:
            nc.sync.dma_start(out=wv, in_=wv_d.rearrange("(a p) c -> p a c", p=128))
            nc.sync.dma_start(out=wvc, in_=wvc_d.rearrange("(a p) c -> p a c", p=128))
            bdiag = projp.tile([128, HL], F32R, name="bdiag")
            nc.sync.dma_start(out=bdiag, in_=bdiag_d)
        emit_qk_chunk(wk, wkc, bk_sb, bkc_sb, kT8, blk * 512)
        if blk >= 2:
            for t in range((blk - 2) * 4, blk * 4 - 4):
                emit_v_tile(t)

    # pad-token constants from biases alone (pad x rows are zero):
    # k_pad = bk*sig(bkc) column, v_pad = bv*sig(bvc) row (+ ones at col 64)
    sigk = sigp.tile([128, 512], F32, name="sig")
    nc.scalar.activation(out=sigk[:, 0:1], in_=bkc_sb, func=AF.Sigmoid)
    kpad = projp.tile([128, 1], F32, name="kpad")
    nc.vector.tensor_tensor(out=kpad, in0=bk_sb, in1=sigk[:, 0:1], op=ALU.mult)
    nc.vector.tensor_scalar(out=padk8, in0=bdiag, scalar1=kpad, scalar2=None,
                            op0=ALU.mult)
    sgp = sigp.tile([128, 512], F32, name="sig")
    nc.scalar.activation(out=sgp[0:1, :], in_=bvc_row.bitcast(F32), func=AF.Sigmoid)
    nc.vector.tensor_tensor(
        out=vt_pad[:, :, 0:VL],
        in0=bv_row.bitcast(F32).rearrange("p (h v) -> p h v", h=HL),
        in1=sgp[0:1, :].rearrange("p (h v) -> p h v", h=HL),
        op=ALU.mult)
    nc.scalar.activation(out=vt_pad[:, :, VL:VL + 1],
                         in_=idr[0:1, 0:HL].unsqueeze(-1),
                         func=AF.Copy, scale=0.0, bias=1.0)
    # denominator ones plane of vt8 via gpsimd memset (SBUF only)
    nc.gpsimd.memset(vt8[:, :, :, VL:VL + 1], 1.0)

    emit_qk_chunk(wq, wqc, bq_sb, bqc_sb, qT8, 0)
    emit_epad_chunk(0)

    # ---------------- injected work for the attention stream ----------------
    inject_q = []                     # closures run one per pair-slot
    for t in range(8, NVT):
        inject_q.append(lambda t=t: emit_v_tile(t))
    for c in range(1, NQR):
        inject_q.append(lambda c=c: emit_qk_chunk(wq, wqc, bq_sb, bqc_sb,
                                                  qT8, c * QR))
        inject_q.append(lambda c=c: emit_epad_chunk(c))

    def inject(n):
        for _ in range(n):
            if inject_q:
                inject_q.pop(0)()

    # ---------------- attention ----------------
    def stage_k(h):
        kh8 = stgk.tile([8, 2, S], FP8, name="kh8")
        nc.sync.dma_start(out=kh8, in_=kT8[16 * h:16 * h + 16, :])
        return kh8

    def stage_qh(qr, h):
        qh8 = stgq.tile([8, 2, QR], FP8, name="qh8")
        sl = slice(qr * QR, (qr + 1) * QR)
        nc.sync.dma_start(out=qh8, in_=qT8[16 * h:16 * h + 16, sl])
        return qh8



    def emit_pair(h, p, kh8, qh8):
        sc = scp.tile([128, 2, 512], F32, name="sc")
        for j in range(2):
            ksl = slice((2 * p + j) * 128, (2 * p + j + 1) * 128)
            nc.tensor.matmul(sc[:, j, :], kh8[:, :, ksl], qh8,
                             start=True, stop=True, perf_mode=DR)
        et8 = etp.tile([128, 2, 512], FP8, name="et8")
        scf = sc.rearrange("p a b -> p (a b)")
        if pick(1040, 1240, act_kind="exp") == "A":
            nc.scalar.activation(out=et8.rearrange("p a b -> p (a b)"),
                                 in_=scf, func=AF.Exp, scale=SCALE)
        else:
            nc.vector.tensor_scalar(
                out=et8.bitcast(I8).rearrange("p a b -> p (a b)"),
                in0=scf, scalar1=LOG2E, scalar2=B_SCH,
                op0=ALU.mult, op1=ALU.add)
        return et8

    def stage_ep(qr, h):
        ep = stge.tile([1, QR], FP8, name="ep")
        nc.sync.dma_start(out=ep, in_=e_pad[h:h + 1, qr, :])
        return ep

    def head_ctx_open(h, ep):
        acc = accp.tile([128, QR], F32, name="acc")
        nc.tensor.matmul(acc[0:VL + 1, :], vt_pad[:, h, :], ep,
                         start=True, stop=False)
        return acc

    def head_loop(qr, h, kh8, qh8, ep, epi, vload):
        acc = None
        pend = []
        for p in range(NPAIR):
            inject(vload)
            if epi is not None:
                next(epi, None)
            et8 = emit_pair(h, p, kh8, qh8)
            pend.append((et8, p))
            if len(pend) > 4:
                if acc is None:
                    acc = head_ctx_open(h, ep)
                _emit_ctx(acc, h, *pend.pop(0))
        for pr in pend:
            if acc is None:
                acc = head_ctx_open(h, ep)
            _emit_ctx(acc, h, *pr)
        return acc

    def _emit_ctx(acc, h, et8, p):
        nc.tensor.matmul(acc[0:VL + 1, :], vt8[:, h, 2 * p:2 * p + 2, 0:VL + 1],
                         et8, start=False, stop=(p == NPAIR - 1), perf_mode=DR)

    def head_epilogue(qr, h, ct, out_sb):
        tp = tpp.tile([128, 4, VL + 1], F32, name="tp")
        for qt in range(4):
            nc.tensor.transpose(
                out=tp[:, qt, :],
                in_=ct[:, qt * 128:(qt + 1) * 128],
                identity=ident[0:VL + 1, 0:VL + 1])
            yield
        rc4 = rcp.tile([128, 4, 1], F32, name="rc")
        nc.vector.reciprocal(out=rc4, in_=tp[:, :, VL:VL + 1])
        busy["D"] += 392
        nc.vector.tensor_tensor(
            out=out_sb[:, :, h * VL:(h + 1) * VL], in0=tp[:, :, 0:VL],
            in1=rc4.to_broadcast([128, 4, VL]), op=ALU.mult)
        if h == HL - 1:
            r0 = qr * QR
            yr = y_d[r0:r0 + QR, :].rearrange("(a p) c -> p a c", p=128)
            nc.sync.dma_start(out=yr, in_=out_sb)
        yield

    # ---- front: scores/exp for ALL of qr0's 8 head-loops, interleaved
    # with the remaining v tiles / qT8 chunks. Heads 0-1 keep their et8 in
    # SBUF; heads 2-7 spill to DRAM (DMA-only cost) and reload for the ctx
    # replay once vt8 is complete. This keeps ACT/DVE fed with exp work
    # through the otherwise PE-bound projection window. ----
    # ---- warmup: scores/exp for (qr0,h0),(qr0,h1) interleaved with v/qk
    # injections; their ctx replays once vt8 is complete ----
    kh8_0 = stage_k(0)
    kh8_1 = stage_k(1)
    qh8_0 = stage_qh(0, 0)
    qh8_1 = stage_qh(0, 1)
    ep_0 = stage_ep(0, 0)
    ep_1 = stage_ep(0, 1)
    stash = {0: [], 1: []}
    for p in range(NPAIR):
        inject(1)
        stash[0].append(emit_pair(0, p, kh8_0, qh8_0))
        inject(1)
        stash[1].append(emit_pair(1, p, kh8_1, qh8_1))
    inject(len(inject_q))
    prev = None                           # (qr, h, ct) awaiting epilogue
    out_sbs = {}
    out_sbs[0] = outp.tile([128, 4, 512], F32, name="out_sb")
    for wh in range(2):
        acc = head_ctx_open(wh, ep_0 if wh == 0 else ep_1)
        for p, et8 in enumerate(stash[wh]):
            _emit_ctx(acc, wh, et8, p)
        ct = ctp.tile([VL + 1, QR], F32, name="ct")
        copy_any(ct, acc[0:VL + 1, :], 512)
        if prev is not None:
            for _ in head_epilogue(prev[0], prev[1], prev[2], out_sbs[0]):
                pass
        prev = (0, wh, ct)
    loops = [(qr, h) for qr in range(NQR) for h in range(HL)][2:]
    kh8_next = stage_k(loops[0][1])
    stq_next = (stage_qh(*loops[0]), stage_ep(*loops[0]))
    for i, (qr, h) in enumerate(loops):
        if h == 0 and qr > 0:
            out_sbs[qr] = outp.tile([128, 4, 512], F32, name="out_sb")
        kh8 = kh8_next
        qh8, ep = stq_next
        if i + 1 < len(loops):
            kh8_next = stage_k(loops[i + 1][1])
            stq_next = (stage_qh(*loops[i + 1]), stage_ep(*loops[i + 1]))
        epi = None
        if prev is not None:
            epi = head_epilogue(prev[0], prev[1], prev[2], out_sbs[prev[0]])
        acc = head_loop(qr, h, kh8, qh8, ep, epi, 0)
        if epi is not None:
            for _ in epi:
                pass
        ct = ctp.tile([VL + 1, QR], F32, name="ct")
        copy_any(ct, acc[0:VL + 1, :], 512)
        prev = (qr, h, ct)
    for _ in head_epilogue(prev[0], prev[1], prev[2], out_sbs[prev[0]]):
        pass

    for p in (tpp, accp, scp, outp, rcp, ctp, stge, stgq, stgk, etp,
              sigp, projp, ph13, ph1, wpool, const):
        p.release()


_NC = None


def _get_nc():
    global _NC
    if _NC is None:
        _NC = _build()
    return _NC


def _shard_inputs(inputs):
    x = np.ascontiguousarray(np.asarray(inputs["x"], dtype=np.float32))
    ident = np.eye(128, dtype=np.float32)
    bdiag = np.repeat(np.eye(8, dtype=np.float32), 16, axis=0)
    in_maps = []
    for c in range(8):
        b, hg = c // 2, c % 2
        qk = slice(hg * 128, (hg + 1) * 128)
        vv = slice(hg * 512, (hg + 1) * 512)
        in_maps.append({
            "x": np.ascontiguousarray(x[b]),
            "idr": ident,
            "bdiag": bdiag,
            "wq": np.ascontiguousarray(inputs["Wq"][:, qk]),
            "wqc": np.ascontiguousarray(inputs["Wqc"][:, qk]),
            "wk": np.ascontiguousarray(inputs["Wk"][:, qk]),
            "wkc": np.ascontiguousarray(inputs["Wkc"][:, qk]),
            "wv": np.ascontiguousarray(inputs["Wv"][:, vv]),
            "wvc": np.ascontiguousarray(inputs["Wvc"][:, vv]),
            "bq": np.ascontiguousarray(inputs["bq"][qk]),
            "bqc": np.ascontiguousarray(inputs["bqc"][qk]),
            "bk": np.ascontiguousarray(inputs["bk"][qk]),
            "bkc": np.ascontiguousarray(inputs["bkc"][qk]),
            "bv": np.ascontiguousarray(inputs["bv"][vv]),
            "bvc": np.ascontiguousarray(inputs["bvc"][vv]),
        })
    return in_maps


def kernel(**inputs) -> np.ndarray:
    nc = _get_nc()
    in_maps = _shard_inputs(inputs)
    res = run_bass_kernel_spmd(nc, in_maps, list(range(8)))
    out = np.empty((B, S, 1024), np.float32)
    for c in range(8):
        b, hg = c // 2, c % 2
        out[b, :, hg * 512:(hg + 1) * 512] = res.results[c]["y"]
    return out


if __name__ == "__main__":
    rng = np.random.default_rng(0)
    d = 1.0 / np.sqrt(D)
    inputs = {
        "x": rng.standard_normal((B, S, D), dtype=np.float32),
        "Wq": rng.standard_normal((D, 256), dtype=np.float32) * d,
        "bq": rng.standard_normal(256).astype(np.float32) * 0.02,
        "Wqc": rng.standard_normal((D, 256), dtype=np.float32) * d,
        "bqc": rng.standard_normal(256).astype(np.float32) * 0.02,
        "Wk": rng.standard_normal((D, 256), dtype=np.float32) * d,
        "bk": rng.standard_normal(256).astype(np.float32) * 0.02,
        "Wkc": rng.standard_normal((D, 256), dtype=np.float32) * d,
        "bkc": rng.standard_normal(256).astype(np.float32) * 0.02,
        "Wv": rng.standard_normal((D, 1024), dtype=np.float32) * d,
        "bv": rng.standard_normal(1024).astype(np.float32) * 0.02,
        "Wvc": rng.standard_normal((D, 1024), dtype=np.float32) * d,
        "bvc": rng.standard_normal(1024).astype(np.float32) * 0.02,
    }
    y = kernel(**inputs)
    print("kernel output", y.shape, y.dtype, float(np.abs(y).max()))


# revision 39
# speedup vs baseline: 1.2616x; 1.0021x over previous
"""Trainium2 Bass kernel for AttentionM (dense transformer block).

Computes, for x [4, 2048, 1024] and q/k/v CSS-gated projections:
    q = (x@Wq+bq)*sigmoid(x@Wqc+bqc)   -> [B, Sp, 16 heads, 16]
    k, v likewise (v 64-wide heads)
    ctx = softmax(q k^T / 8) v          -> [B, S, 1024]
with Sp = S+16 zero-padded rows; pad tokens are bias-only css outputs and
are folded in analytically (no padded x anywhere).

Sharding over 8 NeuronCores: 4-way data parallel over batch x 2-way tensor
parallel over heads (8 heads per core).

Per-core dataflow (low-precision attention, rel err ~1.8e-2 vs 2e-2 budget):
  1. x streams in four 512-token blocks; PE-transposed to feature-major xT
     (f32r, 1.5 cyc/row) and copied to SBUF by whichever of ACT/DVE is
     projected-idler (greedy busy-ns counters steer every flexible op).
  2. k/q projections per 512-token chunk: f32r matmul chains, ACT sigmoid,
     DVE scalar_tensor_tensor writing fp8e4m3 kT8/qT8 directly. v is
     token-major fp8 (vt8 [128, 8h, 16, 80]; 80-stride so DoubleRow
     ldweights sees a %16 interleave step; col 64 = ones denominator via a
     single gpsimd memset); bias via K=1 ones-column matmuls.
  3. Pad tokens: k_pad = bk*sig(bkc), v_pad = bv*sig(bvc) from the bias
     vectors alone (pad x rows are zero). Pad scores for all 8 heads via one
     block-diag fp8 matmul per q-range; e_pad = exp(s/8 + ln16) collapses
     the 16 identical pad rows (fp8, values <= ~22); each ctx accumulation
     opens with an fp8 K=1 rank-1 pad update inside the DoubleRow group.
  4. Attention per (qr in 4 x 512 q, h in 8): scores via DoubleRow fp8
     matmuls at 0.5 cyc/row (kh8/qh8 staged [8,2,*] by one linearizing
     SBUF->SBUF DMA each; interleave lane (p,j) maps head-dim 2p+j on both
     sides). exp on ACT (native Exp -> fp8e4m3 out, exact round-to-nearest)
     or DVE (Schraudolph: one tensor_scalar mult+add -> int8 whose bits ARE
     the e4m3 pattern; round-half-even convert; zero-mean offset 55.54),
     split by projected busy-ns with activation-table switches (Exp vs
     Sigmoid tables) charged 1283ns. ctx accumulates one DoubleRow matmul
     per k-tile pair, trailing exp by 4 pairs.
  5. Epilogue per (qr,h), deferred one head: 4 PE transposes to token-major,
     one DVE reciprocal [128,4,1] of the denominators, one broadcast
     multiply into out_sb; one output DMA per qr range.
  Warmup: scores/exp for (qr0,h0/h1) are emitted interleaved with the
  v-tile / qT8-chunk stream (one injected closure per pair slot) so ACT/DVE
  have exp work during the PE-bound projection window; their ctx replays
  right after. PSUM: 3x[128,2,512] score pairs + acc [128,512] + transpose
  scratch = exactly 8 banks.
"""

import sys

if "/opt/trn_rl_repo" not in sys.path:
    sys.path.insert(0, "/opt/trn_rl_repo")

import numpy as np

import concourse.bacc as bacc
import concourse.mybir as mybir
import concourse.tile as tile
from concourse.bass_utils import run_bass_kernel_spmd
from concourse.masks import make_identity

F32 = mybir.dt.float32
F32R = mybir.dt.float32r
FP8 = mybir.dt.float8e4
I8 = mybir.dt.int8
AF = mybir.ActivationFunctionType
ALU = mybir.AluOpType
DR = mybir.MatmulPerfMode.DoubleRow

B = 4
S = 2048
D = 1024
DC = 8            # contraction chunks of 128
HL = 8            # heads per core
QL = 16           # q/k head dim
VL = 64           # v head dim
NVT = 16          # real-token v tiles (pad handled analytically)
QR = 512          # q range per attention loop
NQR = S // QR     # 4
NPAIR = 8         # k-tile pairs per loop (16 tiles of 128 = 2048 real k)
SCALE = 1.0 / 8.0
LOG2E = float(np.log2(np.e))
B_SCH = 55.54     # zero-mean Schraudolph offset (RHE convert)
VSTR = 80         # padded v-feature stride (%16 == 0 for DoubleRow ldweights)


def _build(repeat=1):
    nc = bacc.Bacc("TRN2", target_bir_lowering=False, debug=False, num_devices=8)

    x_d = nc.dram_tensor("x", [S, D], F32R, kind="ExternalInput").ap()
    idr_d = nc.dram_tensor("idr", [128, 128], F32R, kind="ExternalInput").ap()
    bdiag_d = nc.dram_tensor("bdiag", [128, 8], F32R, kind="ExternalInput").ap()
    wq_d = nc.dram_tensor("wq", [D, 128], F32R, kind="ExternalInput").ap()
    wqc_d = nc.dram_tensor("wqc", [D, 128], F32R, kind="ExternalInput").ap()
    wk_d = nc.dram_tensor("wk", [D, 128], F32R, kind="ExternalInput").ap()
    wkc_d = nc.dram_tensor("wkc", [D, 128], F32R, kind="ExternalInput").ap()
    wv_d = nc.dram_tensor("wv", [D, 512], F32R, kind="ExternalInput").ap()
    wvc_d = nc.dram_tensor("wvc", [D, 512], F32R, kind="ExternalInput").ap()
    bq_d = nc.dram_tensor("bq", [128], F32, kind="ExternalInput").ap()
    bqc_d = nc.dram_tensor("bqc", [128], F32, kind="ExternalInput").ap()
    bk_d = nc.dram_tensor("bk", [128], F32, kind="ExternalInput").ap()
    bkc_d = nc.dram_tensor("bkc", [128], F32, kind="ExternalInput").ap()
    bv_d = nc.dram_tensor("bv", [512], F32R, kind="ExternalInput").ap()
    bvc_d = nc.dram_tensor("bvc", [512], F32R, kind="ExternalInput").ap()
    y_d = nc.dram_tensor("y", [S, 512], F32, kind="ExternalOutput").ap()

    with tile.TileContext(nc) as tc:
        for _ in range(repeat):
            _emit(nc, tc, x_d, idr_d, bdiag_d, wq_d, wqc_d, wk_d, wkc_d, wv_d,
                  wvc_d, bq_d, bqc_d, bk_d, bkc_d, bv_d, bvc_d, y_d)
    nc.compile()
    return nc


def _emit(nc, tc, x_d, idr_d, bdiag_d, wq_d, wqc_d, wk_d, wkc_d, wv_d,
          wvc_d, bq_d, bqc_d, bk_d, bkc_d, bv_d, bvc_d, y_d):
    # ---------------- pools ----------------
    const = tc.alloc_tile_pool(name="const", bufs=1)
    wpool = tc.alloc_tile_pool(name="wpool", bufs=1)
    ph1 = tc.alloc_tile_pool(name="ph1", bufs=3)
    ph13 = tc.alloc_tile_pool(name="ph13", bufs=1)
    projp = tc.alloc_tile_pool(name="projp", bufs=1)
    sigp = tc.alloc_tile_pool(name="sigp", bufs=2)
    etp = tc.alloc_tile_pool(name="etp", bufs=20)
    stgk = tc.alloc_tile_pool(name="stgk", bufs=3)
    stgq = tc.alloc_tile_pool(name="stgq", bufs=2)
    stge = tc.alloc_tile_pool(name="stge", bufs=3)
    ctp = tc.alloc_tile_pool(name="ctp", bufs=2)
    rcp = tc.alloc_tile_pool(name="rcp", bufs=2)
    outp = tc.alloc_tile_pool(name="outp", bufs=2)
    # PSUM: 3x2 + 1 + 1 = 8 banks
    scp = tc.alloc_tile_pool(name="scp", bufs=3, space="PSUM")
    accp = tc.alloc_tile_pool(name="accp", bufs=1, space="PSUM")
    tpp = tc.alloc_tile_pool(name="tpp", bufs=1, space="PSUM")

    ep0s = {}
    # greedy ACT/DVE balance by projected busy-ns; ACT activation-table
    # switches (Exp vs Sigmoid live in different tables) cost 1283ns each
    busy = {"A": 0.0, "D": 0.0}
    act_tbl = [None]

    def act_table(kind):
        if kind in ("exp", "sigmoid") and act_tbl[0] != kind:
            busy["A"] += 1283
            act_tbl[0] = kind

    def pick(act_cost, dve_cost, act_kind=None):
        extra = 1283 if (act_kind in ("exp", "sigmoid")
                         and act_tbl[0] != act_kind) else 0
        if busy["A"] + act_cost + extra <= busy["D"] + dve_cost:
            busy["A"] += act_cost
            if act_kind:
                act_table(act_kind)
            return "A"
        busy["D"] += dve_cost
        return "D"

    def copy_any(out, in_, free):
        if pick(free * 0.833 + 160, free * 1.042 + 200) == "A":
            nc.scalar.activation(out=out, in_=in_, func=AF.Copy)
        else:
            nc.vector.tensor_copy(out=out, in_=in_)

    # ---------------- constants ----------------
    idr = const.tile([128, 128], F32R, name="idr")
    nc.sync.dma_start(out=idr, in_=idr_d)
    ident = const.tile([128, 128], F32)
    make_identity(nc, ident)

    def ppart(bias_d, dtype=F32):
        t = const.tile([128, 1], dtype, name=f"b_{bias_d.name}")
        nc.sync.dma_start(out=t, in_=bias_d.unsqueeze(-1))
        return t

    # ---------------- long-lived tensors ----------------
    xT_parts = [ph13.tile([128, DC, 512], F32R, name=f"xT{i}") for i in range(4)]

    def xTs(d, c0, csz):
        part = c0 // 512
        lo = c0 - part * 512
        return xT_parts[part][:, d, lo:lo + csz]

    wq = wpool.tile([128, DC, 128], F32R, name="wq")
    wqc = wpool.tile([128, DC, 128], F32R, name="wqc")
    wk = wpool.tile([128, DC, 128], F32R, name="wk")
    wkc = wpool.tile([128, DC, 128], F32R, name="wkc")
    wv = wpool.tile([128, DC, 512], F32R, name="wv")
    wvc = wpool.tile([128, DC, 512], F32R, name="wvc")

    qT8 = projp.tile([128, S], FP8, name="qT8")
    kT8 = projp.tile([128, S], FP8, name="kT8")
    vt8 = projp.tile([128, HL, NVT, VSTR], FP8, name="vt8")
    vt_pad = projp.tile([1, HL, VL + 1], FP8, name="vt_pad")
    e_pad = projp.tile([HL, NQR, QR], FP8, name="e_pad")
    padk8 = projp.tile([128, HL], FP8, name="padk8")
    ln16_sb = projp.tile([128, 1], F32, name="ln16")
    ones_col = const.tile([1, 128], F32R, name="ones_col")

    # ---------------- phase emitters ----------------
    def emit_x_tile(t):
        xt = ph1.tile([128, D], F32R, name="xload")
        nc.sync.dma_start(out=xt, in_=x_d[t * 128:(t + 1) * 128, :])
        return xt

    def emit_tposes(t, xt):
        part = t // 4
        lo = t * 128 - part * 512
        tp = scp.tile([128, 2, 512], F32, name="sc")
        tpf = tp.bitcast(F32R).rearrange("p a b -> p (a b)")
        for d in range(DC):
            nc.tensor.transpose(
                out=tpf[:, d * 128:(d + 1) * 128],
                in_=xt[:, d * 128:(d + 1) * 128],
                identity=idr)
        copy_any(xT_parts[part][:, :, lo:lo + 128],
                 tpf.rearrange("p (b c) -> p b c", b=8), 1024)

    def emit_qk_chunk(wl, wcl, bl, bcl, dest8, c0):
        ps = scp.tile([128, 2, 512], F32, name="sc")
        psf = ps.rearrange("p a b -> p (a b)")
        for d in range(DC):
            nc.tensor.matmul(psf[:, 0:512], wl[:, d, :], xTs(d, c0, 512),
                             start=(d == 0), stop=(d == DC - 1))
        for d in range(DC):
            nc.tensor.matmul(psf[:, 512:1024], wcl[:, d, :], xTs(d, c0, 512),
                             start=(d == 0), stop=(d == DC - 1))
        sig = sigp.tile([128, 512], F32, name="sig")
        busy["A"] += 612
        act_table("sigmoid")
        nc.scalar.activation(out=sig, in_=psf[:, 512:1024],
                             func=AF.Sigmoid, bias=bcl)
        busy["D"] += 658
        nc.vector.scalar_tensor_tensor(
            out=dest8[:, c0:c0 + 512], in0=psf[:, 0:512], scalar=bl,
            in1=sig, op0=ALU.add, op1=ALU.mult)

    def emit_v_tile(t):
        tc0 = t * 128
        ps = scp.tile([128, 2, 512], F32, name="sc")
        psf = ps.rearrange("p a b -> p (a b)")
        for d in range(DC):
            nc.tensor.matmul(psf[:, 0:512], xTs(d, tc0, 128), wv[:, d, :],
                             start=(d == 0), stop=False)
        nc.tensor.matmul(psf[:, 0:512], ones_col, bv_row,
                         start=False, stop=True)
        for d in range(DC):
            nc.tensor.matmul(psf[:, 512:1024], xTs(d, tc0, 128), wvc[:, d, :],
                             start=(d == 0), stop=False)
        nc.tensor.matmul(psf[:, 512:1024], ones_col, bvc_row,
                         start=False, stop=True)
        sg = sigp.tile([128, 512], F32, name="sig")
        busy["A"] += 612
        act_table("sigmoid")
        nc.scalar.activation(out=sg, in_=psf[:, 512:1024], func=AF.Sigmoid)
        busy["D"] += 658
        nc.vector.tensor_tensor(
            out=vt8[:, :, t, 0:VL],
            in0=psf[:, 0:512].rearrange("p (h v) -> p h v", h=HL),
            in1=sg.rearrange("p (h v) -> p h v", h=HL),
            op=ALU.mult)

    def emit_epad_chunk(qr):
        ps = scp.tile([128, 2, 512], F32, name="sc")
        nc.tensor.matmul(ps[0:HL, 0, :], padk8,
                         qT8[:, qr * QR:(qr + 1) * QR], start=True, stop=True)
        busy["A"] += 612
        act_table("exp")
        nc.scalar.activation(out=e_pad[:, qr, :], in_=ps[0:HL, 0, :],
                             func=AF.Exp, scale=SCALE, bias=ln16_sb[0:HL, :])


    # ---------------- P0: x stream, transposes, kT8 ----------------
    bias_done = False
    for blk in range(4):
        for t in range(blk * 4, blk * 4 + 4):
            xt = emit_x_tile(t)
            if not bias_done:
                bq_sb = ppart(bq_d)
                bqc_sb = ppart(bqc_d)
                bk_sb = ppart(bk_d)
                bkc_sb = ppart(bkc_d)
                bv_row = const.tile([1, 512], F32R, name="bv_row")
                nc.sync.dma_start(out=bv_row, in_=bv_d.unsqueeze(0))
                bvc_row = const.tile([1, 512], F32R, name="bvc_row")
                nc.sync.dma_start(out=bvc_row, in_=bvc_d.unsqueeze(0))
                nc.scalar.activation(out=ones_col, in_=idr[0:1, :],
                                     func=AF.Copy, scale=0.0, bias=1.0)
                nc.scalar.activation(out=ln16_sb, in_=idr[:, 0:1], func=AF.Copy,
                                     scale=0.0, bias=float(np.log(16.0)))
                bias_done = True
            emit_tposes(t, xt)
        if blk == 0:
            for w_sb, w_dd in ((wk, wk_d), (wkc, wkc_d), (wq, wq_d), (wqc, wqc_d)):
                nc.sync.dma_start(out=w_sb,
                                  in_=w_dd.rearrange("(a p) c -> p a c", p=128))
        if blk == 2:
            nc.sync.dma_start(out=wv, in_=wv_d.rearrange("(a p) c -> p a c", p=128))
            nc.sync.dma_start(out=wvc, in_=wvc_d.rearrange("(a p) c -> p a c", p=128))
            bdiag = projp.tile([128, HL], F32R, name="bdiag")
            nc.sync.dma_start(out=bdiag, in_=bdiag_d)
        emit_qk_chunk(wk, wkc, bk_sb, bkc_sb, kT8, blk * 512)
        if blk >= 2:
            for t in range((blk - 2) * 4, blk * 4 - 4):
                emit_v_tile(t)

    # pad-token constants from biases alone (pad x rows are zero):
    # k_pad = bk*sig(bkc) column, v_pad = bv*sig(bvc) row (+ ones at col 64)
    sigk = sigp.tile([128, 512], F32, name="sig")
    nc.scalar.activation(out=sigk[:, 0:1], in_=bkc_sb, func=AF.Sigmoid)
    kpad = projp.tile([128, 1], F32, name="kpad")
    nc.vector.tensor_tensor(out=kpad, in0=bk_sb, in1=sigk[:, 0:1], op=ALU.mult)
    nc.vector.tensor_scalar(out=padk8, in0=bdiag, scalar1=kpad, scalar2=None,
                            op0=ALU.mult)
    sgp = sigp.tile([128, 512], F32, name="sig")
    nc.scalar.activation(out=sgp[0:1, :], in_=bvc_row.bitcast(F32), func=AF.Sigmoid)
    nc.vector.tensor_tensor(
        out=vt_pad[:, :, 0:VL],
        in0=bv_row.bitcast(F32).rearrange("p (h v) -> p h v", h=HL),
        in1=sgp[0:1, :].rearrange("p (h v) -> p h v", h=HL),
        op=ALU.mult)
    nc.scalar.activation(out=vt_pad[:, :, VL:VL + 1],
                         in_=idr[0:1, 0:HL].unsqueeze(-1),
                         func=AF.Copy, scale=0.0, bias=1.0)
    # denominator ones plane of vt8 via gpsimd memset (SBUF only)
    nc.gpsimd.memset(vt8[:, :, :, VL:VL + 1], 1.0)

    emit_qk_chunk(wq, wqc, bq_sb, bqc_sb, qT8, 0)
    emit_epad_chunk(0)

    # ---------------- injected work for the attention stream ----------------
    inject_q = []                     # closures run one per pair-slot
    for t in range(8, NVT):
        inject_q.append(lambda t=t: emit_v_tile(t))
    inject_q.append(lambda: emit_qk_chunk(wq, wqc, bq_sb, bqc_sb, qT8, QR))
    inject_q.append(lambda: emit_epad_chunk(1))
    late_q = []
    for c in range(2, NQR):
        late_q.append(lambda c=c: emit_qk_chunk(wq, wqc, bq_sb, bqc_sb,
                                                qT8, c * QR))
        late_q.append(lambda c=c: emit_epad_chunk(c))

    def inject(n):
        for _ in range(n):
            if inject_q:
                inject_q.pop(0)()

    # ---------------- attention ----------------
    def stage_k(h):
        kh8 = stgk.tile([8, 2, S], FP8, name="kh8")
        nc.sync.dma_start(out=kh8, in_=kT8[16 * h:16 * h + 16, :])
        return kh8

    def stage_qh(qr, h):
        qh8 = stgq.tile([8, 2, QR], FP8, name="qh8")
        sl = slice(qr * QR, (qr + 1) * QR)
        nc.sync.dma_start(out=qh8, in_=qT8[16 * h:16 * h + 16, sl])
        return qh8



    def emit_pair(h, p, kh8, qh8):
        sc = scp.tile([128, 2, 512], F32, name="sc")
        for j in range(2):
            ksl = slice((2 * p + j) * 128, (2 * p + j + 1) * 128)
            nc.tensor.matmul(sc[:, j, :], kh8[:, :, ksl], qh8,
                             start=True, stop=True, perf_mode=DR)
        et8 = etp.tile([128, 2, 512], FP8, name="et8")
        scf = sc.rearrange("p a b -> p (a b)")
        if pick(1040, 1240, act_kind="exp") == "A":
            nc.scalar.activation(out=et8.rearrange("p a b -> p (a b)"),
                                 in_=scf, func=AF.Exp, scale=SCALE)
        else:
            nc.vector.tensor_scalar(
                out=et8.bitcast(I8).rearrange("p a b -> p (a b)"),
                in0=scf, scalar1=LOG2E, scalar2=B_SCH,
                op0=ALU.mult, op1=ALU.add)
        return et8

    def stage_ep(qr, h):
        ep = stge.tile([1, QR], FP8, name="ep")
        nc.sync.dma_start(out=ep, in_=e_pad[h:h + 1, qr, :])
        return ep

    def head_ctx_open(h, ep):
        acc = accp.tile([128, QR], F32, name="acc")
        nc.tensor.matmul(acc[0:VL + 1, :], vt_pad[:, h, :], ep,
                         start=True, stop=False)
        return acc

    def head_loop(qr, h, kh8, qh8, ep, epi, vload):
        acc = None
        pend = []
        for p in range(NPAIR):
            inject(vload)
            if epi is not None:
                next(epi, None)
            et8 = emit_pair(h, p, kh8, qh8)
            pend.append((et8, p))
            if len(pend) > 4:
                if acc is None:
                    acc = head_ctx_open(h, ep)
                _emit_ctx(acc, h, *pend.pop(0))
        for pr in pend:
            if acc is None:
                acc = head_ctx_open(h, ep)
            _emit_ctx(acc, h, *pr)
        return acc

    def _emit_ctx(acc, h, et8, p):
        nc.tensor.matmul(acc[0:VL + 1, :], vt8[:, h, 2 * p:2 * p + 2, 0:VL + 1],
                         et8, start=False, stop=(p == NPAIR - 1), perf_mode=DR)

    def head_epilogue(qr, h, ct, out_sb):
        tp = tpp.tile([128, 4, VL + 1], F32, name="tp")
        for qt in range(4):
            nc.tensor.transpose(
                out=tp[:, qt, :],
                in_=ct[:, qt * 128:(qt + 1) * 128],
                identity=ident[0:VL + 1, 0:VL + 1])
            yield
        rc4 = rcp.tile([128, 4, 1], F32, name="rc")
        nc.vector.reciprocal(out=rc4, in_=tp[:, :, VL:VL + 1])
        busy["D"] += 392
        nc.vector.tensor_tensor(
            out=out_sb[:, :, h * VL:(h + 1) * VL], in0=tp[:, :, 0:VL],
            in1=rc4.to_broadcast([128, 4, VL]), op=ALU.mult)
        if h == HL - 1:
            r0 = qr * QR
            yr = y_d[r0:r0 + QR, :].rearrange("(a p) c -> p a c", p=128)
            nc.sync.dma_start(out=yr, in_=out_sb)
        yield

    # ---- front: scores/exp for ALL of qr0's 8 head-loops, interleaved
    # with the remaining v tiles / qT8 chunks. Heads 0-1 keep their et8 in
    # SBUF; heads 2-7 spill to DRAM (DMA-only cost) and reload for the ctx
    # replay once vt8 is complete. This keeps ACT/DVE fed with exp work
    # through the otherwise PE-bound projection window. ----
    # ---- warmup: scores/exp for (qr0,h0),(qr0,h1) interleaved with v/qk
    # injections; their ctx replays once vt8 is complete ----
    kh8_0 = stage_k(0)
    kh8_1 = stage_k(1)
    qh8_0 = stage_qh(0, 0)
    qh8_1 = stage_qh(0, 1)
    ep_0 = stage_ep(0, 0)
    ep_1 = stage_ep(0, 1)
    stash = {0: [], 1: []}
    for p in range(NPAIR):
        inject(1)
        stash[0].append(emit_pair(0, p, kh8_0, qh8_0))
        inject(1)
        stash[1].append(emit_pair(1, p, kh8_1, qh8_1))
    inject(len(inject_q))
    inject_q.extend(late_q)
    prev = None                           # (qr, h, ct) awaiting epilogue
    out_sbs = {}
    out_sbs[0] = outp.tile([128, 4, 512], F32, name="out_sb")
    for wh in range(2):
        acc = head_ctx_open(wh, ep_0 if wh == 0 else ep_1)
        for p, et8 in enumerate(stash[wh]):
            _emit_ctx(acc, wh, et8, p)
        ct = ctp.tile([VL + 1, QR], F32, name="ct")
        copy_any(ct, acc[0:VL + 1, :], 512)
        if prev is not None:
            for _ in head_epilogue(prev[0], prev[1], prev[2], out_sbs[0]):
                pass
        prev = (0, wh, ct)
    loops = [(qr, h) for qr in range(NQR) for h in range(HL)][2:]
    kh8_next = stage_k(loops[0][1])
    stq_next = (stage_qh(*loops[0]), stage_ep(*loops[0]))
    for i, (qr, h) in enumerate(loops):
        if h == 0 and qr > 0:
            out_sbs[qr] = outp.tile([128, 4, 512], F32, name="out_sb")
        kh8 = kh8_next
        qh8, ep = stq_next
        if i + 1 < len(loops):
            kh8_next = stage_k(loops[i + 1][1])
            stq_next = (stage_qh(*loops[i + 1]), stage_ep(*loops[i + 1]))
        epi = None
        if prev is not None:
            epi = head_epilogue(prev[0], prev[1], prev[2], out_sbs[prev[0]])
        acc = head_loop(qr, h, kh8, qh8, ep, epi, 1 if inject_q else 0)
        if epi is not None:
            for _ in epi:
                pass
        ct = ctp.tile([VL + 1, QR], F32, name="ct")
        copy_any(ct, acc[0:VL + 1, :], 512)
        prev = (qr, h, ct)
    for _ in head_epilogue(prev[0], prev[1], prev[2], out_sbs[prev[0]]):
        pass

    for p in (tpp, accp, scp, outp, rcp, ctp, stge, stgq, stgk, etp,
              sigp, projp, ph13, ph1, wpool, const):
        p.release()


_NC = None


def _get_nc():
    global _NC
    if _NC is None:
        _NC = _build()
    return _NC


def _shard_inputs(inputs):
    x = np.ascontiguousarray(np.asarray(inputs["x"], dtype=np.float32))
    ident = np.eye(128, dtype=np.float32)
    bdiag = np.repeat(np.eye(8, dtype=np.float32), 16, axis=0)
    in_maps = []
    for c in range(8):
        b, hg = c // 2, c % 2
        qk = slice(hg * 128, (hg + 1) * 128)
        vv = slice(hg * 512, (hg + 1) * 512)
        in_maps.append({
            "x": np.ascontiguousarray(x[b]),
            "idr": ident,
            "bdiag": bdiag,
            "wq": np.ascontiguousarray(inputs["Wq"][:, qk]),
            "wqc": np.ascontiguousarray(inputs["Wqc"][:, qk]),
            "wk": np.ascontiguousarray(inputs["Wk"][:, qk]),
            "wkc": np.ascontiguousarray(inputs["Wkc"][:, qk]),
            "wv": np.ascontiguousarray(inputs["Wv"][:, vv]),
            "wvc": np.ascontiguousarray(inputs["Wvc"][:, vv]),
            "bq": np.ascontiguousarray(inputs["bq"][qk]),
            "bqc": np.ascontiguousarray(inputs["bqc"][qk]),
            "bk": np.ascontiguousarray(inputs["bk"][qk]),
            "bkc": np.ascontiguousarray(inputs["bkc"][qk]),
            "bv": np.ascontiguousarray(inputs["bv"][vv]),
            "bvc": np.ascontiguousarray(inputs["bvc"][vv]),
        })
    return in_maps


def kernel(**inputs) -> np.ndarray:
    nc = _get_nc()
    in_maps = _shard_inputs(inputs)
    res = run_bass_kernel_spmd(nc, in_maps, list(range(8)))
    out = np.empty((B, S, 1024), np.float32)
    for c in range(8):
        b, hg = c // 2, c % 2
        out[b, :, hg * 512:(hg + 1) * 512] = res.results[c]["y"]
    return out


if __name__ == "__main__":
    rng = np.random.default_rng(0)
    d = 1.0 / np.sqrt(D)
    inputs = {
        "x": rng.standard_normal((B, S, D), dtype=np.float32),
        "Wq": rng.standard_normal((D, 256), dtype=np.float32) * d,
        "bq": rng.standard_normal(256).astype(np.float32) * 0.02,
        "Wqc": rng.standard_normal((D, 256), dtype=np.float32) * d,
        "bqc": rng.standard_normal(256).astype(np.float32) * 0.02,
        "Wk": rng.standard_normal((D, 256), dtype=np.float32) * d,
        "bk": rng.standard_normal(256).astype(np.float32) * 0.02,
        "Wkc": rng.standard_normal((D, 256), dtype=np.float32) * d,
        "bkc": rng.standard_normal(256).astype(np.float32) * 0.02,
        "Wv": rng.standard_normal((D, 1024), dtype=np.float32) * d,
        "bv": rng.standard_normal(1024).astype(np.float32) * 0.02,
        "Wvc": rng.standard_normal((D, 1024), dtype=np.float32) * d,
        "bvc": rng.standard_normal(1024).astype(np.float32) * 0.02,
    }
    y = kernel(**inputs)
    print("kernel output", y.shape, y.dtype, float(np.abs(y).max()))
